# revision 38
# baseline (speedup 1.0000x reference)
"""Single-head causal attention (B=4, S=2048, M=H=1024) on 8 Trainium2 cores.

Sharding: core = (batch, half). Each core handles one batch and half its
queries. To balance the causal triangle, query 128-blocks are interleaved
stride-2: core half c owns global q-blocks {c, c+2, ..., c+14}, grouped in
4 chunks of 256 queries; chunk j = global blocks {4j+c, 4j+c+2} and attends
key blocks [0, 4j+4) — the last 4 get data-driven causal masks, so the one
compiled program serves both halves (SPMD).

Fast path (zero qk-bias, no padding, zero v-bias) math, with A = Wq.T@Wk
folded on host (scaled by 32 so fp8/bf16 operands are ~unit variance):
  qh[m2, sq] = (32A).T @ qT          (bf16 matmul; fp8 for query cols 512:)
  scoresT[sk, sq] = ktT.T @ qh       (bf16 chunks 0-1, fp8-DoubleRow 2-3)
  e = exp(scoresT/1024 [- 2])        (ACT; fp8 chunks get -2 bias, cancels)
  Y[m, sq] = sum_kb v_kb.T @ e_kb    (late V-proj: raw v, no projection!)
  out[sq, h] = (Y.T @ (32Wv.T)) / (32*den),  den = sum_k e
The V projection is algebraically moved AFTER the attention-weighted sum,
so the big S*M*H projection runs once per core on [256-query, 1024] Y tiles
instead of all 2048 keys (the old kernel projected all of V on both halves
of every batch - pure duplicated work).

fp8 use is per-query-chunk: early queries attend few keys, so fp8 noise in
their softmax doesn't average out; late chunks attend >=512 keys and the
1/sqrt(n_eff) averaging makes fp8 safe (verified vs reference on host).
"""

import os

import numpy as np

B, S, MD, HD = 4, 2048, 1024, 1024
P = 128
NB = S // P            # 16 key/query blocks per batch
NCH = 4                # q-chunks of 256 per core
SQL = S // 2           # 1024 local queries per core
N_CORES = 8

FP8_CHUNK1 = True      # extend fp8 scores/Y to chunk 1 (queries 256:512)


def _build_fast(fp8_chunk1: bool):
    import concourse.bacc as bacc
    import concourse.mybir as mybir
    import concourse.tile as tile

    f32 = mybir.dt.float32
    bf16 = mybir.dt.bfloat16
    f8 = mybir.dt.float8e4
    Act = mybir.ActivationFunctionType
    DR = mybir.MatmulPerfMode.DoubleRow

    nc = bacc.Bacc("TRN2", num_swdge_queues=4, dynamic_dma_scratch_size=2048)

    nbf = 512 if fp8_chunk1 else 1024  # bf16 key coverage (chunks 0[,1])
    MC = MD // P   # 8 contraction chunks
    NKB = nbf // P
    # All inputs are pre-rearranged on host to partition-major layout, so
    # every load is a plain 2D copy: 128 big contiguous descriptors instead
    # of 1024+ small ones (8x less DMA issue time).
    a16h0 = nc.dram_tensor("a16h0", [P, MC, P], bf16, kind="ExternalInput")
    a16 = nc.dram_tensor("a16", [MC - 1, P, MC, P], bf16,
                         kind="ExternalInput")
    a8 = nc.dram_tensor("a8", [P, MC, HD], f8, kind="ExternalInput")
    qt16 = nc.dram_tensor("qt16", [P, MC, 512], bf16, kind="ExternalInput")
    qt8 = nc.dram_tensor("qt8", [P, MC, 512], f8, kind="ExternalInput")
    kt16 = nc.dram_tensor("kt16", [P, MC, nbf], bf16, kind="ExternalInput")
    kt8 = nc.dram_tensor("kt8", [P, MC, S], f8, kind="ExternalInput")
    v16 = nc.dram_tensor("v16", [P, NKB, MD], bf16, kind="ExternalInput")
    v8 = nc.dram_tensor("v8", [P, NB, MD], f8, kind="ExternalInput")
    wvt16 = nc.dram_tensor("wvt16", [P, MC, HD], bf16, kind="ExternalInput")
    m16 = nc.dram_tensor("m16", [P, 4, 256], bf16, kind="ExternalInput")
    m32 = nc.dram_tensor("m32", [P, 4, 256], f32, kind="ExternalInput")
    out = nc.dram_tensor("out", [SQL, HD], f32, kind="ExternalOutput")

    with tile.TileContext(nc) as tc:
        with (
            tc.tile_pool(name="res", bufs=1) as res,
            tc.tile_pool(name="exp", bufs=10) as epool,
            tc.tile_pool(name="ysb", bufs=2) as ypool,
            tc.tile_pool(name="outp", bufs=2) as outp,
            tc.tile_pool(name="small", bufs=6) as small,
            # PSUM: 8 banks total, every tile is bank-rounded. "y" is a shared
            # ring for q-proj psums, Y-accumulation passes and out-proj tiles.
            tc.tile_pool(name="yp", bufs=4, space="PSUM") as ypp,
            tc.tile_pool(name="sc", bufs=2, space="PSUM") as scp,
            tc.tile_pool(name="dn", bufs=2, space="PSUM") as dnp,
        ):
            # ---- resident tiles + DMA kickoff ----
            # Criticals first on each queue; a16 arrives as per-hb column
            # slices so the first q-proj psum only waits on ~0.75MB. The big
            # fp8 bulk (kt8/v8) sits on the otherwise-idle gpsimd queue
            # behind a tiny SBUF->SBUF DMA that depends on the first q-proj
            # copy, so it cannot crowd the startup-critical transfers.
            # critical path: qt16 + the hb=0 slice of a16 (0.75MB on scalar)
            qt16_t = res.tile([P, MC, 512], bf16, tag="qt16")
            nc.scalar.dma_start(qt16_t[:], qt16.ap())
            a16_t = res.tile([P, MC, HD], bf16, tag="a16")
            nc.scalar.dma_start(a16_t[:, :, 0:P], a16h0.ap())

            mt16 = res.tile([P, 4, 256], bf16, tag="mt16")
            nc.gpsimd.dma_start(mt16[:], m16.ap())
            mt32 = res.tile([P, 4, 256], f32, tag="mt32")
            nc.gpsimd.dma_start(mt32[:], m32.ap())

            qt8_t = res.tile([P, MC, 512], f8, tag="qt8")
            if not fp8_chunk1:
                nc.gpsimd.dma_start(qt8_t[:], qt8.ap())
            v16_t = res.tile([P, NKB, MD], bf16, tag="v16")

            # Remaining resident tiles: loads emitted inside phase 1a (one
            # iteration ahead of use) so the scheduler cannot couple the
            # first matmul chains' semaphore waits to them; bulk loads are
            # additionally serialized behind criticals via tiny SBUF->SBUF
            # "gate" writes into their own target tiles (WAW data deps);
            # the real full-tile load then overwrites the gate bytes.
            kt16_t = res.tile([P, MC, nbf], bf16, tag="kt16")
            a8_t = res.tile([P, MC, HD], f8, tag="a8")
            wv_t = res.tile([P, MC, HD], bf16, tag="wv")
            kt8_t = res.tile([P, MC, S], f8, tag="kt8")
            v8_t = res.tile([P, NB, MD], f8, tag="v8")

            ones16 = res.tile([P, 2], bf16, tag="ones16")
            nc.vector.memset(ones16[:], 32.0)
            ones8 = res.tile([P, 2, 2], f8, tag="ones8")
            nc.vector.memset(ones8[:], 32.0)
            nbias = res.tile([P, 1], f32, tag="nbias")
            nc.vector.memset(nbias[:], -2.0)

            qh16 = res.tile([P, MC, 512], bf16, tag="qh16")
            qh8 = res.tile([P, MC, 512], f8, tag="qh8")
            if fp8_chunk1:
                qh8c1 = res.tile([P, MC, 256], f8, tag="qh8c1")

            # ---- phase 1a: qh cols 0:512 (bf16) ----
            for hb in range(MC):
                if hb + 1 < MC:  # prefetch next a16 slice (4+ gated)
                    h2 = hb + 1
                    if h2 >= 4:
                        nc.sync.dma_start(a16_t[:, 0, h2 * P:h2 * P + 1],
                                          qh16[:, 0, 0:1])
                    nc.sync.dma_start(
                        a16_t[:, :, h2 * P:(h2 + 1) * P], a16.ap()[h2 - 1])
                ps = ypp.tile([P, 512], f32, tag="y", name=f"q16_{hb}")
                for mc in range(MC):
                    nc.tensor.matmul(
                        ps[:], a16_t[:, mc, hb * P:(hb + 1) * P],
                        qt16_t[:, mc, :],
                        start=(mc == 0), stop=(mc == MC - 1))
                nc.vector.tensor_copy(qh16[:, hb, :], ps[:])
                if fp8_chunk1:
                    nc.scalar.copy(qh8c1[:, hb, :], ps[:, 256:512])
                if hb == 0:
                    # gate the non-critical loads behind the first q-proj
                    # copy (real data deps); v8 behind the kt8 transfer
                    gsrc = qh8c1 if fp8_chunk1 else qh8
                    if fp8_chunk1:
                        nc.gpsimd.dma_start(qt8_t[:, 0, 0:2],
                                            gsrc[:, 0, 0:2])
                        nc.gpsimd.dma_start(qt8_t[:], qt8.ap())
                    nc.gpsimd.dma_start(v16_t[:, 0, 0:2], qh16[:, 0, 0:2])
                    nc.gpsimd.dma_start(v16_t[:], v16.ap())
                    nc.gpsimd.dma_start(kt8_t[:, 0, 0:2], gsrc[:, 0, 0:2])
                    nc.gpsimd.dma_start(kt8_t[:], kt8.ap())
                    nc.gpsimd.dma_start(v8_t[:, 0, 0:2], kt8_t[:, 0, 0:2])
                    nc.gpsimd.dma_start(v8_t[:], v8.ap())
                if hb == 2:
                    nc.scalar.dma_start(kt16_t[:, 0, 0:2], qt16_t[:, 0, 0:2])
                    nc.scalar.dma_start(kt16_t[:], kt16.ap())
                    nc.scalar.dma_start(a8_t[:, 0, 0:2], qt8_t[:, 0, 0:2])
                    nc.scalar.dma_start(a8_t[:], a8.ap())
                if hb == 4:
                    nc.sync.dma_start(wv_t[:, 0, 0:2], kt16_t[:, 0, 0:2])
                    nc.sync.dma_start(wv_t[:], wvt16.ap())

            # ---- phase 1b: qh cols 512:1024 (fp8 DoubleRow) ----
            for hb in range(MC):
                ps = ypp.tile([P, 512], f32, tag="y", name=f"q8_{hb}")
                for i in range(4):
                    nc.tensor.matmul(
                        ps[:], a8_t[:, 2 * i:2 * i + 2, hb * P:(hb + 1) * P],
                        qt8_t[:, 2 * i:2 * i + 2, :],
                        start=(i == 0), stop=(i == 3), perf_mode=DR)
                nc.scalar.copy(qh8[:, hb, :], ps[:])

            # ---- phase 2: attention chunks ----
            pending = None  # (ysb, col-offset, rr[2], j) awaiting out-proj

            def emit_out_proj(p):
                ysb, off, drs, j = p
                for t in range(2):
                    ops = [ypp.tile([P, 512], f32, tag="y",
                                    name=f"op{j}_{t}_{hc}") for hc in range(2)]
                    for mb in range(MC):
                        for hc in range(2):
                            nc.tensor.matmul(
                                ops[hc][:],
                                ysb[:, mb, off + t * P:off + (t + 1) * P],
                                wv_t[:, mb, hc * 512:(hc + 1) * 512],
                                start=(mb == 0), stop=(mb == MC - 1))
                    rr = drs[t][:, 1:2]
                    o = outp.tile([P, HD], f32, tag="o")
                    lb = 2 * j + t
                    nc.vector.tensor_scalar_mul(o[:, 0:512], ops[0][:], rr[:])
                    nc.sync.dma_start(out.ap()[lb * P:(lb + 1) * P, 0:512],
                                      o[:, 0:512])
                    nc.scalar.activation(o[:, 512:1024], ops[1][:], Act.Copy,
                                         scale=rr[:])
                    nc.sync.dma_start(out.ap()[lb * P:(lb + 1) * P, 512:1024],
                                      o[:, 512:1024])

            for j in range(2):
                E = 4 * j + 4
                use8 = (j >= 2) or (j == 1 and fp8_chunk1)
                sq0 = (j % 2) * 256  # col offset within qh16/qh8 halves
                dns = [dnp.tile([P, 2], f32, tag="d", name=f"d{j}_{t}")
                       for t in range(2)]
                exps = []

                if not use8:
                    qrhs = qh16[:, :, sq0:sq0 + 256]
                    for kb in range(E):
                        sps = scp.tile([P, 256], f32, tag="s")
                        for mc in range(MC):
                            nc.tensor.matmul(
                                sps[:], kt16_t[:, mc, kb * P:(kb + 1) * P],
                                qrhs[:, mc, :],
                                start=(mc == 0), stop=(mc == MC - 1))
                        ex = epool.tile([P, 256], bf16, tag="e")
                        nc.scalar.activation(ex[:], sps[:], Act.Exp,
                                             scale=1.0 / 1024.0)
                        if kb >= 4 * j:
                            nc.vector.tensor_mul(ex[:], ex[:],
                                                 mt16[:, kb - 4 * j, :])
                        for t in range(2):
                            nc.tensor.matmul(
                                dns[t][:], ex[:, t * P:(t + 1) * P],
                                ones16[:], start=(kb == 0), stop=(kb == E - 1))
                        exps.append(ex)
                        if kb == 1 and pending is not None:
                            emit_out_proj(pending)
                            pending = None
                else:
                    if j == 1:
                        qrhs = qh8c1
                    else:
                        qrhs = qh8[:, :, sq0:sq0 + 256]
                    NP = E // 2
                    for p in range(NP):
                        e8p = epool.tile([P, 2, 256], f8, tag="e8")
                        for s2 in range(2):
                            kb = 2 * p + s2
                            sps = scp.tile([P, 256], f32, tag="s")
                            for i in range(4):
                                nc.tensor.matmul(
                                    sps[:],
                                    kt8_t[:, 2 * i:2 * i + 2,
                                          kb * P:(kb + 1) * P],
                                    qrhs[:, 2 * i:2 * i + 2, :],
                                    start=(i == 0), stop=(i == 3),
                                    perf_mode=DR)
                            if kb >= 4 * j:
                                nc.vector.tensor_add(sps[:], sps[:],
                                                     mt32[:, kb - 4 * j, :])
                            nc.scalar.activation(e8p[:, s2, :], sps[:],
                                                 Act.Exp, scale=1.0 / 1024.0,
                                                 bias=nbias[:])
                        for t in range(2):
                            nc.tensor.matmul(
                                dns[t][:], e8p[:, :, t * P:(t + 1) * P],
                                ones8[:], start=(p == 0), stop=(p == NP - 1),
                                perf_mode=DR)
                        exps.append(e8p)
                        if p == 0 and pending is not None:
                            emit_out_proj(pending)
                            pending = None

                # denominators -> reciprocal (frees dn ring before Y passes)
                drs = []
                for t in range(2):
                    dr = small.tile([P, 2], f32, tag="dr")
                    nc.vector.tensor_copy(dr[:, 0:1], dns[t][:, 0:1])
                    nc.vector.reciprocal(dr[:, 1:2], dr[:, 0:1])
                    drs.append(dr)

                # Y accumulation in 4 passes of 2 m-blocks (PSUM bank limit)
                ysb = ypool.tile([P, MC, 256], bf16, tag="ysb")
                for d in range(4):
                    ys2 = [ypp.tile([P, 256], f32, tag="y",
                                    name=f"y{j}_{d}_{i}") for i in range(2)]
                    for i in range(2):
                        mb = 2 * d + i
                        if not use8:
                            for kb in range(E):
                                nc.tensor.matmul(
                                    ys2[i][:],
                                    v16_t[:, kb, mb * P:(mb + 1) * P],
                                    exps[kb][:],
                                    start=(kb == 0), stop=(kb == E - 1))
                        else:
                            NP = E // 2
                            for p in range(NP):
                                nc.tensor.matmul(
                                    ys2[i][:],
                                    v8_t[:, 2 * p:2 * p + 2,
                                         mb * P:(mb + 1) * P],
                                    exps[p][:],
                                    start=(p == 0), stop=(p == NP - 1),
                                    perf_mode=DR)
                        if mb % 2 == 0:
                            nc.vector.tensor_copy(ysb[:, mb, :], ys2[i][:])
                        else:
                            nc.scalar.copy(ysb[:, mb, :], ys2[i][:])
                pending = (ysb, 0, drs, j)

            # ---- merged chunks 2 & 3 (both fp8, local queries 512:1024) ----
            # Shared 512-wide scores for kb<12 (both chunks attend them) and
            # one shared Y accumulation in [128,512] psums; kb 12-15 are
            # chunk-3-only and accumulate into the upper half.
            exps23 = []
            for p in range(8):
                wide = p < 6
                cols = 512 if wide else 256
                et = epool.tile([P, 2, cols], f8,
                                tag="e8w" if wide else "e8", name=f"e23_{p}")
                for s2 in range(2):
                    kb = 2 * p + s2
                    sps = scp.tile([P, cols], f32, tag="s", name=f"s23_{kb}")
                    qrhs = qh8[:, :, 0:512] if wide else qh8[:, :, 256:512]
                    for i in range(4):
                        nc.tensor.matmul(
                            sps[:], kt8_t[:, 2 * i:2 * i + 2,
                                          kb * P:(kb + 1) * P],
                            qrhs[:, 2 * i:2 * i + 2, :],
                            start=(i == 0), stop=(i == 3), perf_mode=DR)
                    if wide and kb >= 8:
                        nc.vector.tensor_add(sps[:, 0:256], sps[:, 0:256],
                                             mt32[:, kb - 8, :])
                    if not wide:
                        nc.vector.tensor_add(sps[:], sps[:],
                                             mt32[:, kb - 12, :])
                    nc.scalar.activation(et[:, s2, :], sps[:], Act.Exp,
                                         scale=1.0 / 1024.0, bias=nbias[:])
                exps23.append(et)
                if p == 0 and pending is not None:
                    emit_out_proj(pending)
                    pending = None

            # denominators: chunk2 over pairs 0-5 (cols 0:256), chunk3 over
            # all pairs (cols 256:512 of wide tiles, 0:256 of narrow)
            all_drs = []
            for cj, off, NPd in ((2, 256, 6), (3, 0, 8)):
                dns = [dnp.tile([P, 2], f32, tag="d", name=f"d{cj}_{t}")
                       for t in range(2)]
                for p in range(NPd if cj == 2 else 8):
                    for t in range(2):
                        if cj == 2:
                            lhs = exps23[p][:, :, t * P:(t + 1) * P]
                        elif p < 6:
                            lhs = exps23[p][:, :, 256 + t * P:256 + (t + 1) * P]
                        else:
                            lhs = exps23[p][:, :, t * P:(t + 1) * P]
                        nc.tensor.matmul(
                            dns[t][:], lhs, ones8[:],
                            start=(p == 0),
                            stop=(p == (5 if cj == 2 else 7)),
                            perf_mode=DR)
                drs = []
                for t in range(2):
                    dr = small.tile([P, 2], f32, tag="dr")
                    nc.vector.tensor_copy(dr[:, 0:1], dns[t][:, 0:1])
                    nc.vector.reciprocal(dr[:, 1:2], dr[:, 0:1])
                    drs.append(dr)
                all_drs.append(drs)

            # shared Y accumulation: full-width for pairs 0-5, upper half
            # only for the chunk-3 tail pairs 6-7
            ysb23 = ypool.tile([P, MC, 512], bf16, tag="ysb")
            for d in range(4):
                ys2 = [ypp.tile([P, 512], f32, tag="y",
                                name=f"y23_{d}_{i}") for i in range(2)]
                for i in range(2):
                    mb = 2 * d + i
                    for p in range(8):
                        if p < 6:
                            nc.tensor.matmul(
                                ys2[i][:],
                                v8_t[:, 2 * p:2 * p + 2, mb * P:(mb + 1) * P],
                                exps23[p][:], start=(p == 0), stop=False,
                                perf_mode=DR, skip_group_check=True)
                        else:
                            nc.tensor.matmul(
                                ys2[i][:, 256:512],
                                v8_t[:, 2 * p:2 * p + 2, mb * P:(mb + 1) * P],
                                exps23[p][:], start=False, stop=(p == 7),
                                perf_mode=DR, skip_group_check=True)
                    if mb % 2 == 0:
                        nc.vector.tensor_copy(ysb23[:, mb, :], ys2[i][:])
                    else:
                        nc.scalar.copy(ysb23[:, mb, :], ys2[i][:])

            emit_out_proj((ysb23, 0, all_drs[0], 2))
            emit_out_proj((ysb23, 256, all_drs[1], 3))

    nc.compile()
    return nc


def _build_general(use_pad: bool, use_vbias: bool):
    import concourse.bacc as bacc
    import concourse.mybir as mybir
    import concourse.tile as tile

    f32 = mybir.dt.float32
    f32r = mybir.dt.float32r
    bf16 = mybir.dt.bfloat16
    Act = mybir.ActivationFunctionType

    nc = bacc.Bacc("TRN2", num_swdge_queues=4, dynamic_dma_scratch_size=2048)

    qt = nc.dram_tensor("qt", [MD, SQL], f32r, kind="ExternalInput")
    kt = nc.dram_tensor("kt", [MD, S], f32r, kind="ExternalInput")
    vt = nc.dram_tensor("vt", [MD, S], f32r, kind="ExternalInput")
    wqt = nc.dram_tensor("wqt", [MD, HD], f32r, kind="ExternalInput")
    wkt = nc.dram_tensor("wkt", [MD, HD], f32r, kind="ExternalInput")
    wvt = nc.dram_tensor("wvt", [MD, HD], f32r, kind="ExternalInput")
    bq = nc.dram_tensor("bq", [HD], f32, kind="ExternalInput")
    bk = nc.dram_tensor("bk", [HD], f32, kind="ExternalInput")
    masks = nc.dram_tensor("masks", [4, P, 256], bf16, kind="ExternalInput")
    if use_pad:
        padm = nc.dram_tensor("padm", [P, NB], f32, kind="ExternalInput")
    if use_vbias:
        bv = nc.dram_tensor("bv", [HD], f32, kind="ExternalInput")
    out = nc.dram_tensor("out", [SQL, HD], f32, kind="ExternalOutput")

    MC = MD // P   # 8 contraction chunks
    HB = HD // P   # 8 h-blocks (partition dim of qhT/khT)

    with tile.TileContext(nc) as tc:
        with (
            tc.tile_pool(name="res", bufs=1) as res,
            tc.tile_pool(name="w", bufs=10) as wpool,
            tc.tile_pool(name="xin", bufs=4) as xin,
            tc.tile_pool(name="exp", bufs=16) as epool,
            tc.tile_pool(name="outp", bufs=1) as outp,
            tc.tile_pool(name="small", bufs=2) as small,
            tc.tile_pool(name="mm", bufs=5, space="PSUM") as mmp,
            tc.tile_pool(name="sc", bufs=2, space="PSUM") as scp,
            tc.tile_pool(name="dn", bufs=1, space="PSUM") as dnp,
        ):
            qh = res.tile([P, HB, SQL], f32r, tag="qh")
            kh = res.tile([P, HB, S], f32r, tag="kh")
            vh = res.tile([P, NB, HD], bf16, tag="vh")
            mt = res.tile([P, 4, 256], bf16, tag="mt")
            nc.scalar.dma_start(mt[:], masks.ap().rearrange("i p n -> p i n"))
            ones = res.tile([P, 2], bf16, tag="ones")
            nc.vector.memset(ones[:], 1.0)
            bias_t = res.tile([P, 2 * HB], f32, tag="bias")
            bqt = bias_t[:, 0:HB]
            nc.gpsimd.dma_start(bqt[:], bq.ap().rearrange("(hb p) -> p hb", p=P))
            bkt = bias_t[:, HB:2 * HB]
            nc.gpsimd.dma_start(bkt[:], bk.ap().rearrange("(hb p) -> p hb", p=P))
            if use_pad:
                pad_t = res.tile([P, NB], f32, tag="pad")
                nc.gpsimd.dma_start(pad_t[:], padm.ap())
            if use_vbias:
                ones_row = res.tile([1, P], f32r, tag="or")
                bvr = res.tile([1, HD], f32r, tag="bvr")
                nc.gpsimd.memset(ones_row[:].bitcast(f32), 1.0)
                nc.gpsimd.dma_start(bvr[:], bv.ap()[None, :])

            def load_w(dram, split=True):
                tiles = []
                for mc in range(MC):
                    t = wpool.tile([P, HD], f32r, tag="w", name=f"w{mc}")
                    weng = nc.scalar if (mc % 2 == 0 or not split) else nc.sync
                    weng.dma_start(t[:], dram.ap()[mc * P:(mc + 1) * P, :])
                    tiles.append(t)
                return tiles

            class XPair:
                def __init__(self, a, b):
                    self.a, self.b = a, b

                def __getitem__(self, key):
                    _, mc, cols = key
                    t = self.a if mc < 4 else self.b
                    return t[:, mc % 4, cols]

            def load_x(dram, c0):
                r = dram.ap().rearrange("(mc p) s -> p mc s", p=P)
                a = xin.tile([P, 4, 512], f32r, tag="x", name="xa")
                nc.sync.dma_start(a[:], r[:, 0:4, c0:c0 + 512])
                b = xin.tile([P, 4, 512], f32r, tag="x", name="xb")
                nc.sync.dma_start(b[:], r[:, 4:8, c0:c0 + 512])
                return XPair(a, b)

            # ---- Q projection: qh[:, hb, sq] (h on partitions) ----
            wq_t = load_w(wqt, split=False)
            for sqc in range(SQL // 512):
                xts = load_x(qt, sqc * 512)
                for hb in range(HB):
                    ps = mmp.tile([P, 512], f32, tag="mm")
                    for mc in range(MC):
                        nc.tensor.matmul(
                            ps[:], wq_t[mc][:, hb * P:(hb + 1) * P], xts[:, mc, :],
                            start=(mc == 0), stop=(mc == MC - 1))
                    nc.vector.tensor_scalar_add(
                        qh[:, hb, sqc * 512:(sqc + 1) * 512], ps[:],
                        bqt[:, hb:hb + 1])

            # ---- K projection: kh[:, hb, sk] ----
            wk_t = load_w(wkt)
            for skc in range(S // 512):
                xts = load_x(kt, skc * 512)
                for hb in range(HB):
                    ps = mmp.tile([P, 512], f32, tag="mm")
                    for mc in range(MC):
                        nc.tensor.matmul(
                            ps[:], wk_t[mc][:, hb * P:(hb + 1) * P], xts[:, mc, :],
                            start=(mc == 0), stop=(mc == MC - 1))
                    nc.vector.tensor_scalar_add(
                        kh[:, hb, skc * 512:(skc + 1) * 512], ps[:],
                        bkt[:, hb:hb + 1])

            # ---- V projection: vh[:, skb, h] (keys on partitions) ----
            wv_t = load_w(wvt)
            for skc in range(S // 512):
                xts = load_x(vt, skc * 512)
                for sbl in range(4):
                    skb = skc * 4 + sbl
                    for hc in range(2):
                        ps = mmp.tile([P, 512], f32, tag="mm")
                        for mc in range(MC):
                            nc.tensor.matmul(
                                ps[:], xts[:, mc, sbl * P:(sbl + 1) * P],
                                wv_t[mc][:, hc * 512:(hc + 1) * 512],
                                start=(mc == 0),
                                stop=(mc == MC - 1) and not use_vbias)
                        if use_vbias:
                            nc.tensor.matmul(
                                ps[:], ones_row[:],
                                bvr[:, hc * 512:(hc + 1) * 512],
                                start=False, stop=True)
                        nc.vector.tensor_copy(vh[:, skb, hc * 512:(hc + 1) * 512], ps[:])


            # ---- attention, chunk j = 256 queries, keys [0, (4j+4)*128) ----
            for j in range(NCH):
                E = 4 * j + 4
                sq0 = j * 256
                exps = []
                for kb in range(E):
                    sps = scp.tile([P, 256], f32, tag="s")
                    for hb in range(HB):
                        nc.tensor.matmul(
                            sps[:], kh[:, hb, kb * P:(kb + 1) * P],
                            qh[:, hb, sq0:sq0 + 256],
                            start=(hb == 0), stop=(hb == HB - 1))
                    ex = epool.tile([P, 256], bf16, tag="e")
                    nc.scalar.activation(ex[:], sps[:], Act.Exp, scale=1.0 / 32.0)
                    if kb >= 4 * j:
                        nc.vector.tensor_mul(ex[:], ex[:], mt[:, kb - 4 * j, :])
                    if use_pad:
                        nc.vector.tensor_scalar_mul(ex[:], ex[:], pad_t[:, kb:kb + 1])
                    exps.append(ex)

                for t in range(2):
                    dps = dnp.tile([P, 2], f32, tag="d")
                    avs = [mmp.tile([P, 512], f32, tag="mm", name=f"av{j}_{t}_{hc2}")
                           for hc2 in range(2)]
                    for kb in range(E):
                        lhs = exps[kb][:, t * P:(t + 1) * P]
                        for hc in range(2):
                            nc.tensor.matmul(
                                avs[hc][:], lhs, vh[:, kb, hc * 512:(hc + 1) * 512],
                                start=(kb == 0), stop=(kb == E - 1))
                        nc.tensor.matmul(
                            dps[:], lhs, ones[:],
                            start=(kb == 0), stop=(kb == E - 1))
                    dr = small.tile([P, 2], f32, tag="dr")
                    nc.vector.tensor_copy(dr[:, 0:1], dps[:, 0:1])
                    rr = dr[:, 1:2]
                    nc.vector.reciprocal(rr[:], dr[:, 0:1])
                    o = outp.tile([P, HD], f32, tag="o")
                    for hc in range(2):
                        nc.vector.tensor_scalar_mul(
                            o[:, hc * 512:(hc + 1) * 512], avs[hc][:], rr[:])
                    lb = 2 * j + t
                    nc.sync.dma_start(out.ap()[lb * P:(lb + 1) * P, :], o[:])

    nc.compile()
    return nc


def _ntff_hook():
    """NTFF profile hook via direct ctypes into libaxon_pjrt.so (the
    antenv.axon_hooks module is absent in this image). Dev-only: guarded
    by ATTN_PROF_DIR in kernel(); the grading path never reaches this."""
    import contextlib
    import ctypes
    import sys

    lib = ctypes.CDLL("/opt/axon/libaxon_pjrt.so")
    lib.axon_start_nrt_profile.argtypes = [
        ctypes.POINTER(ctypes.c_int64), ctypes.c_size_t]
    lib.axon_start_nrt_profile.restype = ctypes.c_int64
    lib.axon_stop_nrt_profile.argtypes = [ctypes.c_char_p]
    lib.axon_stop_nrt_profile.restype = ctypes.c_int64

    @contextlib.contextmanager
    def _hook(output_dir, device_ids):
        import jax
        jax.devices()
        if device_ids:
            ids = (ctypes.c_int64 * len(device_ids))(*device_ids)
            rc = lib.axon_start_nrt_profile(ids, len(device_ids))
        else:
            rc = lib.axon_start_nrt_profile(None, 0)
        if rc != 0:
            raise RuntimeError(f"axon_start_nrt_profile rc={rc}")
        try:
            yield
        finally:
            n = lib.axon_stop_nrt_profile(str(output_dir).encode())
            print(f"profile: {n} file(s) written to {output_dir}",
                  file=sys.stderr)

    return _hook


def _run(nc, in_maps):
    from concourse.bass_utils import run_bass_kernel_spmd

    prof_dir = os.environ.get("ATTN_PROF_DIR")
    if prof_dir:
        hook = _ntff_hook()
        with hook(prof_dir, [0]):
            return run_bass_kernel_spmd(nc, in_maps, list(range(N_CORES)))
    return run_bass_kernel_spmd(nc, in_maps, list(range(N_CORES)))


def _perms():
    perms = []
    for c in range(2):
        perm = np.concatenate([
            np.arange(P) + (4 * j + c + 2 * t) * P
            for j in range(NCH) for t in range(2)
        ])
        perms.append(perm)
    return perms


def _masks(ml_dtypes):
    """Causal masks for the 4 tail key-blocks of each chunk, per half c.
    entry [i, a, col]: key (4j+i)*128+a vs query (4j+c+2t)*128+b, t=col//128."""
    mask_b, mask_f = [], []
    a = np.arange(P)[:, None]
    col = np.arange(256)[None, :]
    for c in range(2):
        t = col // P
        b_ = col % P
        m = np.stack([
            (128 * i + a <= 128 * (c + 2 * t) + b_) for i in range(4)
        ])
        mask_b.append(m.astype(np.float32).astype(ml_dtypes.bfloat16))
        mask_f.append(np.where(m, np.float32(0), np.float32(-1e9)))
    return mask_b, mask_f


def kernel(q, k, v, attention_mask, Wq_w, Wq_b, Wk_w, Wk_b, Wv_w, Wv_b):
    import ml_dtypes

    q = np.asarray(q, dtype=np.float32)
    k = np.asarray(k, dtype=np.float32)
    v = np.asarray(v, dtype=np.float32)
    am = np.asarray(attention_mask)

    use_pad = not bool((am == 1).all())
    use_vbias = bool(np.any(np.asarray(Wv_b) != 0))
    use_qkbias = bool(np.any(np.asarray(Wq_b) != 0) or np.any(np.asarray(Wk_b) != 0))

    perms = _perms()

    if use_qkbias or use_pad or use_vbias:
        return _kernel_general(q, k, v, am, Wq_w, Wq_b, Wk_w, Wk_b, Wv_w,
                               Wv_b, use_pad, use_vbias, perms)

    f8 = ml_dtypes.float8_e4m3
    bf = ml_dtypes.bfloat16

    nc = _build_fast(FP8_CHUNK1)

    def pmajor(x):
        """[MC*P, cols] -> [P, MC, cols]: [p, mc, col] = x[mc*128+p, col]."""
        mc = x.shape[0] // P
        return np.ascontiguousarray(
            x.reshape(mc, P, x.shape[1]).transpose(1, 0, 2))

    A32 = (np.asarray(Wq_w, np.float64).T @ np.asarray(Wk_w, np.float64))
    A32 = np.ascontiguousarray((A32 * 32.0).astype(np.float32))
    # a16f[hb, p, mc, j] = A32[mc*128+p, hb*128+j]: per-hb contiguous slices
    a16f = np.ascontiguousarray(
        A32.reshape(8, P, 8, P).transpose(2, 1, 0, 3)).astype(bf)
    a16h0 = np.ascontiguousarray(a16f[0])
    a16 = np.ascontiguousarray(a16f[1:])
    a8 = pmajor(A32).astype(f8)
    wvt16 = pmajor(np.ascontiguousarray(
        (np.asarray(Wv_w, np.float32).T * 32.0))).astype(bf)

    mask_b, mask_f = _masks(ml_dtypes)
    # masks to [p, i, n]
    mask_b = [np.ascontiguousarray(m.transpose(1, 0, 2)) for m in mask_b]
    mask_f = [np.ascontiguousarray(m.transpose(1, 0, 2)) for m in mask_f]

    nbf = 512 if FP8_CHUNK1 else 1024
    in_maps = []
    for cid in range(N_CORES):
        b, c = cid // 2, cid % 2
        qT = np.ascontiguousarray(q[b].T[:, perms[c]])
        kT = np.ascontiguousarray(k[b].T)
        vkb = v[b].reshape(NB, P, MD).transpose(1, 0, 2)  # [p, kb, m]
        m = dict(
            a16h0=a16h0, a16=a16, a8=a8,
            qt16=pmajor(qT[:, 0:512]).astype(bf),
            qt8=pmajor(qT[:, 512:1024]).astype(f8),
            kt16=pmajor(kT[:, 0:nbf]).astype(bf),
            kt8=pmajor(kT).astype(f8),
            v16=np.ascontiguousarray(vkb[:, 0:nbf // P, :]).astype(bf),
            v8=np.ascontiguousarray(vkb).astype(f8),
            wvt16=wvt16,
            m16=mask_b[c], m32=mask_f[c],
        )
        in_maps.append(m)

    res = _run(nc, in_maps)

    out = np.empty((B, S, HD), np.float32)
    for cid in range(N_CORES):
        b, c = cid // 2, cid % 2
        out[b, perms[c], :] = res.results[cid]["out"]
    return out


def _kernel_general(q, k, v, am, Wq_w, Wq_b, Wk_w, Wk_b, Wv_w, Wv_b,
                    use_pad, use_vbias, perms):
    nc = _build_general(use_pad, use_vbias)

    wqt = np.ascontiguousarray(np.asarray(Wq_w, np.float32).T)
    wkt = np.ascontiguousarray(np.asarray(Wk_w, np.float32).T)
    wvt = np.ascontiguousarray(np.asarray(Wv_w, np.float32).T)
    bq = np.ascontiguousarray(np.asarray(Wq_b, np.float32))
    bk = np.ascontiguousarray(np.asarray(Wk_b, np.float32))
    bv = np.ascontiguousarray(np.asarray(Wv_b, np.float32))

    import ml_dtypes
    mask_b, _ = _masks(ml_dtypes)

    kT = [np.ascontiguousarray(k[b].T) for b in range(B)]
    vT = [np.ascontiguousarray(v[b].T) for b in range(B)]

    in_maps = []
    for cid in range(N_CORES):
        b, c = cid // 2, cid % 2
        qT = np.ascontiguousarray(q[b].T[:, perms[c]])
        m = dict(qt=qT, kt=kT[b], vt=vT[b], wqt=wqt, wkt=wkt, wvt=wvt,
                 bq=bq, bk=bk, masks=mask_b[c])
        if use_pad:
            padv = am[b].astype(np.float32)
            m["padm"] = np.ascontiguousarray(padv.reshape(NB, P).T)
        if use_vbias:
            m["bv"] = bv
        in_maps.append(m)

    res = _run(nc, in_maps)

    out = np.empty((B, S, HD), np.float32)
    for cid in range(N_CORES):
        b, c = cid // 2, cid % 2
        out[b, perms[c], :] = res.results[cid]["out"]
    return out


# revision 41
# speedup vs baseline: 1.1641x; 1.1641x over previous
"""Single-head causal attention (B=4, S=2048, M=H=1024) on 8 Trainium2 cores.

Sharding: core = (batch, half). Each core handles one batch and half its
queries. To balance the causal triangle, query 128-blocks are interleaved
stride-2: core half c owns global q-blocks {c, c+2, ..., c+14}, grouped in
4 chunks of 256 queries; chunk j = global blocks {4j+c, 4j+c+2} and attends
key blocks [0, 4j+4) — the last 4 get data-driven causal masks, so the one
compiled program serves both halves (SPMD).

Fast path (zero qk-bias, no padding, zero v-bias) math, with A = Wq.T@Wk
folded on host (scaled by 32 so fp8/bf16 operands are ~unit variance):
  qh[m2, sq] = (32A).T @ qT          (bf16 matmul; fp8 for query cols 512:)
  scoresT[sk, sq] = ktT.T @ qh       (bf16 chunks 0-1, fp8-DoubleRow 2-3)
  e = exp(scoresT/1024 [- 2])        (ACT; fp8 chunks get -2 bias, cancels)
  Y[m, sq] = sum_kb v_kb.T @ e_kb    (late V-proj: raw v, no projection!)
  out[sq, h] = (Y.T @ (32Wv.T)) / (32*den),  den = sum_k e
The V projection is algebraically moved AFTER the attention-weighted sum,
so the big S*M*H projection runs once per core on [256-query, 1024] Y tiles
instead of all 2048 keys (the old kernel projected all of V on both halves
of every batch - pure duplicated work).

fp8 use is per-query-chunk: early queries attend few keys, so fp8 noise in
their softmax doesn't average out; late chunks attend >=512 keys and the
1/sqrt(n_eff) averaging makes fp8 safe (verified vs reference on host).
"""

import os

import numpy as np

B, S, MD, HD = 4, 2048, 1024, 1024
P = 128
NB = S // P            # 16 key/query blocks per batch
NCH = 4                # q-chunks of 256 per core
SQL = S // 2           # 1024 local queries per core
N_CORES = 8

FP8_CHUNK1 = True      # extend fp8 scores/Y to chunk 1 (queries 256:512)


def _build_fast(fp8_chunk1: bool):
    import concourse.bacc as bacc
    import concourse.mybir as mybir
    import concourse.tile as tile

    f32 = mybir.dt.float32
    bf16 = mybir.dt.bfloat16
    f8 = mybir.dt.float8e4
    Act = mybir.ActivationFunctionType
    DR = mybir.MatmulPerfMode.DoubleRow

    nc = bacc.Bacc("TRN2", num_swdge_queues=4, dynamic_dma_scratch_size=2048)

    nbf = 512 if fp8_chunk1 else 1024  # bf16 key coverage (chunks 0[,1])
    MC = MD // P   # 8 contraction chunks
    NKB = nbf // P
    # All inputs are pre-rearranged on host to partition-major layout, so
    # every load is a plain 2D copy: 128 big contiguous descriptors instead
    # of 1024+ small ones (8x less DMA issue time).
    a16h0 = nc.dram_tensor("a16h0", [P, MC, P], bf16, kind="ExternalInput")
    a16 = nc.dram_tensor("a16", [MC - 1, P, MC, P], bf16,
                         kind="ExternalInput")
    a8 = nc.dram_tensor("a8", [P, MC, HD], f8, kind="ExternalInput")
    qt16 = nc.dram_tensor("qt16", [P, MC, 512], bf16, kind="ExternalInput")
    qt8 = nc.dram_tensor("qt8", [P, MC, 512], f8, kind="ExternalInput")
    kt16 = nc.dram_tensor("kt16", [P, MC, nbf], bf16, kind="ExternalInput")
    kt8 = nc.dram_tensor("kt8", [P, MC, S], f8, kind="ExternalInput")
    v16 = nc.dram_tensor("v16", [P, NKB, MD], bf16, kind="ExternalInput")
    v8 = nc.dram_tensor("v8", [P, NB, MD], f8, kind="ExternalInput")
    wvt16 = nc.dram_tensor("wvt16", [P, MC, HD], bf16, kind="ExternalInput")
    m16 = nc.dram_tensor("m16", [P, 4, 256], bf16, kind="ExternalInput")
    m32 = nc.dram_tensor("m32", [P, 4, 256], f32, kind="ExternalInput")
    out = nc.dram_tensor("out", [SQL, HD], f32, kind="ExternalOutput")

    with tile.TileContext(nc) as tc:
        with (
            tc.tile_pool(name="res", bufs=1) as res,
            tc.tile_pool(name="exp", bufs=10) as epool,
            tc.tile_pool(name="ysb", bufs=2) as ypool,
            tc.tile_pool(name="outp", bufs=2) as outp,
            tc.tile_pool(name="small", bufs=6) as small,
            # PSUM: 8 banks total, every tile is bank-rounded. "y" is a shared
            # ring for q-proj psums, Y-accumulation passes and out-proj tiles.
            tc.tile_pool(name="yp", bufs=4, space="PSUM") as ypp,
            tc.tile_pool(name="sc", bufs=2, space="PSUM") as scp,
            tc.tile_pool(name="dn", bufs=2, space="PSUM") as dnp,
        ):
            # ---- resident tiles + DMA kickoff ----
            # Criticals first on each queue; a16 arrives as per-hb column
            # slices so the first q-proj psum only waits on ~0.75MB. The big
            # fp8 bulk (kt8/v8) sits on the otherwise-idle gpsimd queue
            # behind a tiny SBUF->SBUF DMA that depends on the first q-proj
            # copy, so it cannot crowd the startup-critical transfers.
            # critical path: qt16 + the hb=0 slice of a16 (0.75MB on scalar)
            qt16_t = res.tile([P, MC, 512], bf16, tag="qt16")
            nc.scalar.dma_start(qt16_t[:], qt16.ap())
            a16_t = res.tile([P, MC, HD], bf16, tag="a16")
            nc.scalar.dma_start(a16_t[:, :, 0:P], a16h0.ap())

            mt16 = res.tile([P, 4, 256], bf16, tag="mt16")
            nc.sync.dma_start(mt16[:], m16.ap())
            mt32 = res.tile([P, 4, 256], f32, tag="mt32")
            nc.sync.dma_start(mt32[:], m32.ap())

            qt8_t = res.tile([P, MC, 512], f8, tag="qt8")
            nc.sync.dma_start(qt8_t[:], qt8.ap())
            v16_t = res.tile([P, NKB, MD], bf16, tag="v16")

            # Remaining resident tiles: loads emitted inside phase 1a (one
            # iteration ahead of use) so the scheduler cannot couple the
            # first matmul chains' semaphore waits to them; bulk loads are
            # additionally serialized behind criticals via tiny SBUF->SBUF
            # "gate" writes into their own target tiles (WAW data deps);
            # the real full-tile load then overwrites the gate bytes.
            kt16_t = res.tile([P, MC, nbf], bf16, tag="kt16")
            a8_t = res.tile([P, MC, HD], f8, tag="a8")
            wv_t = res.tile([P, MC, HD], bf16, tag="wv")
            kt8_t = res.tile([P, MC, S], f8, tag="kt8")
            v8_t = res.tile([P, NB, MD], f8, tag="v8")

            ones16 = res.tile([P, 2], bf16, tag="ones16")
            nc.vector.memset(ones16[:], 32.0)
            ones8 = res.tile([P, 2, 2], f8, tag="ones8")
            nc.vector.memset(ones8[:], 32.0)
            nbias = res.tile([P, 1], f32, tag="nbias")
            nc.vector.memset(nbias[:], -2.0)

            qh16 = res.tile([P, MC, 512], bf16, tag="qh16")
            qh8 = res.tile([P, MC, 512], f8, tag="qh8")
            if fp8_chunk1:
                qh8c1 = res.tile([P, MC, 256], f8, tag="qh8c1")

            # ---- phase 1a: qh cols 0:512 (bf16) ----
            for hb in range(MC):
                if hb + 1 < MC:  # prefetch next a16 slice
                    h2 = hb + 1
                    nc.sync.dma_start(
                        a16_t[:, :, h2 * P:(h2 + 1) * P], a16.ap()[h2 - 1])
                ps = ypp.tile([P, 512], f32, tag="y", name=f"q16_{hb}")
                for mc in range(MC):
                    nc.tensor.matmul(
                        ps[:], a16_t[:, mc, hb * P:(hb + 1) * P],
                        qt16_t[:, mc, :],
                        start=(mc == 0), stop=(mc == MC - 1))
                nc.vector.tensor_copy(qh16[:, hb, :], ps[:])
                if fp8_chunk1:
                    nc.scalar.copy(qh8c1[:, hb, :], ps[:, 256:512])
                if hb == 0:
                    # gate fp8 bulk behind the first q-proj copy (real dep),
                    # v8 behind the kt8 transfer, v16 behind a later chain
                    gsrc = qh8c1 if fp8_chunk1 else qh8
                    nc.gpsimd.dma_start(v16_t[:, 0, 0:2], qh16[:, 2, 0:2])
                    nc.gpsimd.dma_start(v16_t[:], v16.ap())
                    nc.gpsimd.dma_start(kt8_t[:, 0, 0:2], gsrc[:, 0, 0:2])
                    nc.gpsimd.dma_start(kt8_t[:], kt8.ap())
                    nc.gpsimd.dma_start(v8_t[:, 0, 0:2], kt8_t[:, 0, 0:2])
                    nc.gpsimd.dma_start(v8_t[:], v8.ap())
                if hb == 2:
                    nc.scalar.dma_start(kt16_t[:, 0, 0:2], qt16_t[:, 0, 0:2])
                    nc.scalar.dma_start(kt16_t[:], kt16.ap())
                    nc.scalar.dma_start(a8_t[:, 0, 0:2], qt8_t[:, 0, 0:2])
                    nc.scalar.dma_start(a8_t[:], a8.ap())
                if hb == 4:
                    nc.sync.dma_start(wv_t[:, 0, 0:2], kt16_t[:, 0, 0:2])
                    nc.sync.dma_start(wv_t[:], wvt16.ap())

            # ---- phase 1b: qh cols 512:1024 (fp8 DoubleRow) ----
            for hb in range(MC):
                ps = ypp.tile([P, 512], f32, tag="y", name=f"q8_{hb}")
                for i in range(4):
                    nc.tensor.matmul(
                        ps[:], a8_t[:, 2 * i:2 * i + 2, hb * P:(hb + 1) * P],
                        qt8_t[:, 2 * i:2 * i + 2, :],
                        start=(i == 0), stop=(i == 3), perf_mode=DR)
                nc.scalar.copy(qh8[:, hb, :], ps[:])

            # ---- phase 2: attention chunks ----
            pending = None  # (ysb, col-offset, rr[2], j) awaiting out-proj

            def emit_out_proj(p):
                ysb, off, drs, j = p
                for t in range(2):
                    ops = [ypp.tile([P, 512], f32, tag="y",
                                    name=f"op{j}_{t}_{hc}") for hc in range(2)]
                    for mb in range(MC):
                        for hc in range(2):
                            nc.tensor.matmul(
                                ops[hc][:],
                                ysb[:, mb, off + t * P:off + (t + 1) * P],
                                wv_t[:, mb, hc * 512:(hc + 1) * 512],
                                start=(mb == 0), stop=(mb == MC - 1))
                    rr = drs[t][:, 1:2]
                    o = outp.tile([P, HD], f32, tag="o")
                    lb = 2 * j + t
                    nc.vector.tensor_scalar_mul(o[:, 0:512], ops[0][:], rr[:])
                    nc.sync.dma_start(out.ap()[lb * P:(lb + 1) * P, 0:512],
                                      o[:, 0:512])
                    nc.scalar.activation(o[:, 512:1024], ops[1][:], Act.Copy,
                                         scale=rr[:])
                    nc.sync.dma_start(out.ap()[lb * P:(lb + 1) * P, 512:1024],
                                      o[:, 512:1024])

            for j in range(2):
                E = 4 * j + 4
                use8 = (j >= 2) or (j == 1 and fp8_chunk1)
                sq0 = (j % 2) * 256  # col offset within qh16/qh8 halves
                dns = [dnp.tile([P, 2], f32, tag="d", name=f"d{j}_{t}")
                       for t in range(2)]
                exps = []

                if not use8:
                    qrhs = qh16[:, :, sq0:sq0 + 256]
                    for kb in range(E):
                        sps = scp.tile([P, 256], f32, tag="s")
                        for mc in range(MC):
                            nc.tensor.matmul(
                                sps[:], kt16_t[:, mc, kb * P:(kb + 1) * P],
                                qrhs[:, mc, :],
                                start=(mc == 0), stop=(mc == MC - 1))
                        ex = epool.tile([P, 256], bf16, tag="e")
                        nc.scalar.activation(ex[:], sps[:], Act.Exp,
                                             scale=1.0 / 1024.0)
                        if kb >= 4 * j:
                            nc.vector.tensor_mul(ex[:], ex[:],
                                                 mt16[:, kb - 4 * j, :])
                        for t in range(2):
                            nc.tensor.matmul(
                                dns[t][:], ex[:, t * P:(t + 1) * P],
                                ones16[:], start=(kb == 0), stop=(kb == E - 1))
                        exps.append(ex)
                        if kb == 1 and pending is not None:
                            emit_out_proj(pending)
                            pending = None
                else:
                    if j == 1:
                        qrhs = qh8c1
                    else:
                        qrhs = qh8[:, :, sq0:sq0 + 256]
                    NP = E // 2
                    for p in range(NP):
                        e8p = epool.tile([P, 2, 256], f8, tag="e8")
                        for s2 in range(2):
                            kb = 2 * p + s2
                            sps = scp.tile([P, 256], f32, tag="s")
                            for i in range(4):
                                nc.tensor.matmul(
                                    sps[:],
                                    kt8_t[:, 2 * i:2 * i + 2,
                                          kb * P:(kb + 1) * P],
                                    qrhs[:, 2 * i:2 * i + 2, :],
                                    start=(i == 0), stop=(i == 3),
                                    perf_mode=DR)
                            if kb >= 4 * j:
                                nc.vector.tensor_add(sps[:], sps[:],
                                                     mt32[:, kb - 4 * j, :])
                            nc.scalar.activation(e8p[:, s2, :], sps[:],
                                                 Act.Exp, scale=1.0 / 1024.0,
                                                 bias=nbias[:])
                        for t in range(2):
                            nc.tensor.matmul(
                                dns[t][:], e8p[:, :, t * P:(t + 1) * P],
                                ones8[:], start=(p == 0), stop=(p == NP - 1),
                                perf_mode=DR)
                        exps.append(e8p)
                        if p == 0 and pending is not None:
                            emit_out_proj(pending)
                            pending = None

                # denominators -> reciprocal (frees dn ring before Y passes)
                drs = []
                for t in range(2):
                    dr = small.tile([P, 2], f32, tag="dr")
                    nc.vector.tensor_copy(dr[:, 0:1], dns[t][:, 0:1])
                    nc.vector.reciprocal(dr[:, 1:2], dr[:, 0:1])
                    drs.append(dr)

                # Y accumulation in 4 passes of 2 m-blocks (PSUM bank limit)
                ysb = ypool.tile([P, MC, 256], bf16, tag="ysb")
                for d in range(4):
                    ys2 = [ypp.tile([P, 256], f32, tag="y",
                                    name=f"y{j}_{d}_{i}") for i in range(2)]
                    for i in range(2):
                        mb = 2 * d + i
                        if not use8:
                            for kb in range(E):
                                nc.tensor.matmul(
                                    ys2[i][:],
                                    v16_t[:, kb, mb * P:(mb + 1) * P],
                                    exps[kb][:],
                                    start=(kb == 0), stop=(kb == E - 1))
                        else:
                            NP = E // 2
                            for p in range(NP):
                                nc.tensor.matmul(
                                    ys2[i][:],
                                    v8_t[:, 2 * p:2 * p + 2,
                                         mb * P:(mb + 1) * P],
                                    exps[p][:],
                                    start=(p == 0), stop=(p == NP - 1),
                                    perf_mode=DR)
                        if mb % 2 == 0:
                            nc.vector.tensor_copy(ysb[:, mb, :], ys2[i][:])
                        else:
                            nc.scalar.copy(ysb[:, mb, :], ys2[i][:])
                pending = (ysb, 0, drs, j)

            # ---- merged chunks 2 & 3 (both fp8, local queries 512:1024) ----
            # Shared 512-wide scores for kb<12 (both chunks attend them) and
            # one shared Y accumulation in [128,512] psums; kb 12-15 are
            # chunk-3-only and accumulate into the upper half.
            exps23 = []
            for p in range(8):
                wide = p < 6
                cols = 512 if wide else 256
                et = epool.tile([P, 2, cols], f8,
                                tag="e8w" if wide else "e8", name=f"e23_{p}")
                for s2 in range(2):
                    kb = 2 * p + s2
                    sps = scp.tile([P, cols], f32, tag="s", name=f"s23_{kb}")
                    qrhs = qh8[:, :, 0:512] if wide else qh8[:, :, 256:512]
                    for i in range(4):
                        nc.tensor.matmul(
                            sps[:], kt8_t[:, 2 * i:2 * i + 2,
                                          kb * P:(kb + 1) * P],
                            qrhs[:, 2 * i:2 * i + 2, :],
                            start=(i == 0), stop=(i == 3), perf_mode=DR)
                    if wide and kb >= 8:
                        nc.vector.tensor_add(sps[:, 0:256], sps[:, 0:256],
                                             mt32[:, kb - 8, :])
                    if not wide:
                        nc.vector.tensor_add(sps[:], sps[:],
                                             mt32[:, kb - 12, :])
                    nc.scalar.activation(et[:, s2, :], sps[:], Act.Exp,
                                         scale=1.0 / 1024.0, bias=nbias[:])
                exps23.append(et)
                if p == 0 and pending is not None:
                    emit_out_proj(pending)
                    pending = None

            # denominators: chunk2 over pairs 0-5 (cols 0:256), chunk3 over
            # all pairs (cols 256:512 of wide tiles, 0:256 of narrow)
            all_drs = []
            for cj, off, NPd in ((2, 256, 6), (3, 0, 8)):
                dns = [dnp.tile([P, 2], f32, tag="d", name=f"d{cj}_{t}")
                       for t in range(2)]
                for p in range(NPd if cj == 2 else 8):
                    for t in range(2):
                        if cj == 2:
                            lhs = exps23[p][:, :, t * P:(t + 1) * P]
                        elif p < 6:
                            lhs = exps23[p][:, :, 256 + t * P:256 + (t + 1) * P]
                        else:
                            lhs = exps23[p][:, :, t * P:(t + 1) * P]
                        nc.tensor.matmul(
                            dns[t][:], lhs, ones8[:],
                            start=(p == 0),
                            stop=(p == (5 if cj == 2 else 7)),
                            perf_mode=DR)
                drs = []
                for t in range(2):
                    dr = small.tile([P, 2], f32, tag="dr")
                    nc.vector.tensor_copy(dr[:, 0:1], dns[t][:, 0:1])
                    nc.vector.reciprocal(dr[:, 1:2], dr[:, 0:1])
                    drs.append(dr)
                all_drs.append(drs)

            # shared Y accumulation: full-width for pairs 0-5, upper half
            # only for the chunk-3 tail pairs 6-7
            ysb23 = ypool.tile([P, MC, 512], bf16, tag="ysb")
            for d in range(4):
                ys2 = [ypp.tile([P, 512], f32, tag="y",
                                name=f"y23_{d}_{i}") for i in range(2)]
                for i in range(2):
                    mb = 2 * d + i
                    for p in range(8):
                        if p < 6:
                            nc.tensor.matmul(
                                ys2[i][:],
                                v8_t[:, 2 * p:2 * p + 2, mb * P:(mb + 1) * P],
                                exps23[p][:], start=(p == 0), stop=False,
                                perf_mode=DR, skip_group_check=True)
                        else:
                            nc.tensor.matmul(
                                ys2[i][:, 256:512],
                                v8_t[:, 2 * p:2 * p + 2, mb * P:(mb + 1) * P],
                                exps23[p][:], start=False, stop=(p == 7),
                                perf_mode=DR, skip_group_check=True)
                    if mb % 2 == 0:
                        nc.vector.tensor_copy(ysb23[:, mb, :], ys2[i][:])
                    else:
                        nc.scalar.copy(ysb23[:, mb, :], ys2[i][:])

            emit_out_proj((ysb23, 0, all_drs[0], 2))
            emit_out_proj((ysb23, 256, all_drs[1], 3))

    nc.compile()
    return nc


def _build_general(use_pad: bool, use_vbias: bool):
    import concourse.bacc as bacc
    import concourse.mybir as mybir
    import concourse.tile as tile

    f32 = mybir.dt.float32
    f32r = mybir.dt.float32r
    bf16 = mybir.dt.bfloat16
    Act = mybir.ActivationFunctionType

    nc = bacc.Bacc("TRN2", num_swdge_queues=4, dynamic_dma_scratch_size=2048)

    qt = nc.dram_tensor("qt", [MD, SQL], f32r, kind="ExternalInput")
    kt = nc.dram_tensor("kt", [MD, S], f32r, kind="ExternalInput")
    vt = nc.dram_tensor("vt", [MD, S], f32r, kind="ExternalInput")
    wqt = nc.dram_tensor("wqt", [MD, HD], f32r, kind="ExternalInput")
    wkt = nc.dram_tensor("wkt", [MD, HD], f32r, kind="ExternalInput")
    wvt = nc.dram_tensor("wvt", [MD, HD], f32r, kind="ExternalInput")
    bq = nc.dram_tensor("bq", [HD], f32, kind="ExternalInput")
    bk = nc.dram_tensor("bk", [HD], f32, kind="ExternalInput")
    masks = nc.dram_tensor("masks", [4, P, 256], bf16, kind="ExternalInput")
    if use_pad:
        padm = nc.dram_tensor("padm", [P, NB], f32, kind="ExternalInput")
    if use_vbias:
        bv = nc.dram_tensor("bv", [HD], f32, kind="ExternalInput")
    out = nc.dram_tensor("out", [SQL, HD], f32, kind="ExternalOutput")

    MC = MD // P   # 8 contraction chunks
    HB = HD // P   # 8 h-blocks (partition dim of qhT/khT)

    with tile.TileContext(nc) as tc:
        with (
            tc.tile_pool(name="res", bufs=1) as res,
            tc.tile_pool(name="w", bufs=10) as wpool,
            tc.tile_pool(name="xin", bufs=4) as xin,
            tc.tile_pool(name="exp", bufs=16) as epool,
            tc.tile_pool(name="outp", bufs=1) as outp,
            tc.tile_pool(name="small", bufs=2) as small,
            tc.tile_pool(name="mm", bufs=5, space="PSUM") as mmp,
            tc.tile_pool(name="sc", bufs=2, space="PSUM") as scp,
            tc.tile_pool(name="dn", bufs=1, space="PSUM") as dnp,
        ):
            qh = res.tile([P, HB, SQL], f32r, tag="qh")
            kh = res.tile([P, HB, S], f32r, tag="kh")
            vh = res.tile([P, NB, HD], bf16, tag="vh")
            mt = res.tile([P, 4, 256], bf16, tag="mt")
            nc.scalar.dma_start(mt[:], masks.ap().rearrange("i p n -> p i n"))
            ones = res.tile([P, 2], bf16, tag="ones")
            nc.vector.memset(ones[:], 1.0)
            bias_t = res.tile([P, 2 * HB], f32, tag="bias")
            bqt = bias_t[:, 0:HB]
            nc.gpsimd.dma_start(bqt[:], bq.ap().rearrange("(hb p) -> p hb", p=P))
            bkt = bias_t[:, HB:2 * HB]
            nc.gpsimd.dma_start(bkt[:], bk.ap().rearrange("(hb p) -> p hb", p=P))
            if use_pad:
                pad_t = res.tile([P, NB], f32, tag="pad")
                nc.gpsimd.dma_start(pad_t[:], padm.ap())
            if use_vbias:
                ones_row = res.tile([1, P], f32r, tag="or")
                bvr = res.tile([1, HD], f32r, tag="bvr")
                nc.gpsimd.memset(ones_row[:].bitcast(f32), 1.0)
                nc.gpsimd.dma_start(bvr[:], bv.ap()[None, :])

            def load_w(dram, split=True):
                tiles = []
                for mc in range(MC):
                    t = wpool.tile([P, HD], f32r, tag="w", name=f"w{mc}")
                    weng = nc.scalar if (mc % 2 == 0 or not split) else nc.sync
                    weng.dma_start(t[:], dram.ap()[mc * P:(mc + 1) * P, :])
                    tiles.append(t)
                return tiles

            class XPair:
                def __init__(self, a, b):
                    self.a, self.b = a, b

                def __getitem__(self, key):
                    _, mc, cols = key
                    t = self.a if mc < 4 else self.b
                    return t[:, mc % 4, cols]

            def load_x(dram, c0):
                r = dram.ap().rearrange("(mc p) s -> p mc s", p=P)
                a = xin.tile([P, 4, 512], f32r, tag="x", name="xa")
                nc.sync.dma_start(a[:], r[:, 0:4, c0:c0 + 512])
                b = xin.tile([P, 4, 512], f32r, tag="x", name="xb")
                nc.sync.dma_start(b[:], r[:, 4:8, c0:c0 + 512])
                return XPair(a, b)

            # ---- Q projection: qh[:, hb, sq] (h on partitions) ----
            wq_t = load_w(wqt, split=False)
            for sqc in range(SQL // 512):
                xts = load_x(qt, sqc * 512)
                for hb in range(HB):
                    ps = mmp.tile([P, 512], f32, tag="mm")
                    for mc in range(MC):
                        nc.tensor.matmul(
                            ps[:], wq_t[mc][:, hb * P:(hb + 1) * P], xts[:, mc, :],
                            start=(mc == 0), stop=(mc == MC - 1))
                    nc.vector.tensor_scalar_add(
                        qh[:, hb, sqc * 512:(sqc + 1) * 512], ps[:],
                        bqt[:, hb:hb + 1])

            # ---- K projection: kh[:, hb, sk] ----
            wk_t = load_w(wkt)
            for skc in range(S // 512):
                xts = load_x(kt, skc * 512)
                for hb in range(HB):
                    ps = mmp.tile([P, 512], f32, tag="mm")
                    for mc in range(MC):
                        nc.tensor.matmul(
                            ps[:], wk_t[mc][:, hb * P:(hb + 1) * P], xts[:, mc, :],
                            start=(mc == 0), stop=(mc == MC - 1))
                    nc.vector.tensor_scalar_add(
                        kh[:, hb, skc * 512:(skc + 1) * 512], ps[:],
                        bkt[:, hb:hb + 1])

            # ---- V projection: vh[:, skb, h] (keys on partitions) ----
            wv_t = load_w(wvt)
            for skc in range(S // 512):
                xts = load_x(vt, skc * 512)
                for sbl in range(4):
                    skb = skc * 4 + sbl
                    for hc in range(2):
                        ps = mmp.tile([P, 512], f32, tag="mm")
                        for mc in range(MC):
                            nc.tensor.matmul(
                                ps[:], xts[:, mc, sbl * P:(sbl + 1) * P],
                                wv_t[mc][:, hc * 512:(hc + 1) * 512],
                                start=(mc == 0),
                                stop=(mc == MC - 1) and not use_vbias)
                        if use_vbias:
                            nc.tensor.matmul(
                                ps[:], ones_row[:],
                                bvr[:, hc * 512:(hc + 1) * 512],
                                start=False, stop=True)
                        nc.vector.tensor_copy(vh[:, skb, hc * 512:(hc + 1) * 512], ps[:])


            # ---- attention, chunk j = 256 queries, keys [0, (4j+4)*128) ----
            for j in range(NCH):
                E = 4 * j + 4
                sq0 = j * 256
                exps = []
                for kb in range(E):
                    sps = scp.tile([P, 256], f32, tag="s")
                    for hb in range(HB):
                        nc.tensor.matmul(
                            sps[:], kh[:, hb, kb * P:(kb + 1) * P],
                            qh[:, hb, sq0:sq0 + 256],
                            start=(hb == 0), stop=(hb == HB - 1))
                    ex = epool.tile([P, 256], bf16, tag="e")
                    nc.scalar.activation(ex[:], sps[:], Act.Exp, scale=1.0 / 32.0)
                    if kb >= 4 * j:
                        nc.vector.tensor_mul(ex[:], ex[:], mt[:, kb - 4 * j, :])
                    if use_pad:
                        nc.vector.tensor_scalar_mul(ex[:], ex[:], pad_t[:, kb:kb + 1])
                    exps.append(ex)

                for t in range(2):
                    dps = dnp.tile([P, 2], f32, tag="d")
                    avs = [mmp.tile([P, 512], f32, tag="mm", name=f"av{j}_{t}_{hc2}")
                           for hc2 in range(2)]
                    for kb in range(E):
                        lhs = exps[kb][:, t * P:(t + 1) * P]
                        for hc in range(2):
                            nc.tensor.matmul(
                                avs[hc][:], lhs, vh[:, kb, hc * 512:(hc + 1) * 512],
                                start=(kb == 0), stop=(kb == E - 1))
                        nc.tensor.matmul(
                            dps[:], lhs, ones[:],
                            start=(kb == 0), stop=(kb == E - 1))
                    dr = small.tile([P, 2], f32, tag="dr")
                    nc.vector.tensor_copy(dr[:, 0:1], dps[:, 0:1])
                    rr = dr[:, 1:2]
                    nc.vector.reciprocal(rr[:], dr[:, 0:1])
                    o = outp.tile([P, HD], f32, tag="o")
                    for hc in range(2):
                        nc.vector.tensor_scalar_mul(
                            o[:, hc * 512:(hc + 1) * 512], avs[hc][:], rr[:])
                    lb = 2 * j + t
                    nc.sync.dma_start(out.ap()[lb * P:(lb + 1) * P, :], o[:])

    nc.compile()
    return nc


def _ntff_hook():
    """NTFF profile hook via direct ctypes into libaxon_pjrt.so (the
    antenv.axon_hooks module is absent in this image). Dev-only: guarded
    by ATTN_PROF_DIR in kernel(); the grading path never reaches this."""
    import contextlib
    import ctypes
    import sys

    lib = ctypes.CDLL("/opt/axon/libaxon_pjrt.so")
    lib.axon_start_nrt_profile.argtypes = [
        ctypes.POINTER(ctypes.c_int64), ctypes.c_size_t]
    lib.axon_start_nrt_profile.restype = ctypes.c_int64
    lib.axon_stop_nrt_profile.argtypes = [ctypes.c_char_p]
    lib.axon_stop_nrt_profile.restype = ctypes.c_int64

    @contextlib.contextmanager
    def _hook(output_dir, device_ids):
        import jax
        jax.devices()
        if device_ids:
            ids = (ctypes.c_int64 * len(device_ids))(*device_ids)
            rc = lib.axon_start_nrt_profile(ids, len(device_ids))
        else:
            rc = lib.axon_start_nrt_profile(None, 0)
        if rc != 0:
            raise RuntimeError(f"axon_start_nrt_profile rc={rc}")
        try:
            yield
        finally:
            n = lib.axon_stop_nrt_profile(str(output_dir).encode())
            print(f"profile: {n} file(s) written to {output_dir}",
                  file=sys.stderr)

    return _hook


def _run(nc, in_maps):
    from concourse.bass_utils import run_bass_kernel_spmd

    prof_dir = os.environ.get("ATTN_PROF_DIR")
    if prof_dir:
        hook = _ntff_hook()
        with hook(prof_dir, [0]):
            return run_bass_kernel_spmd(nc, in_maps, list(range(N_CORES)))
    return run_bass_kernel_spmd(nc, in_maps, list(range(N_CORES)))


def _perms():
    perms = []
    for c in range(2):
        perm = np.concatenate([
            np.arange(P) + (4 * j + c + 2 * t) * P
            for j in range(NCH) for t in range(2)
        ])
        perms.append(perm)
    return perms


def _masks(ml_dtypes):
    """Causal masks for the 4 tail key-blocks of each chunk, per half c.
    entry [i, a, col]: key (4j+i)*128+a vs query (4j+c+2t)*128+b, t=col//128."""
    mask_b, mask_f = [], []
    a = np.arange(P)[:, None]
    col = np.arange(256)[None, :]
    for c in range(2):
        t = col // P
        b_ = col % P
        m = np.stack([
            (128 * i + a <= 128 * (c + 2 * t) + b_) for i in range(4)
        ])
        mask_b.append(m.astype(np.float32).astype(ml_dtypes.bfloat16))
        mask_f.append(np.where(m, np.float32(0), np.float32(-1e9)))
    return mask_b, mask_f


def kernel(q, k, v, attention_mask, Wq_w, Wq_b, Wk_w, Wk_b, Wv_w, Wv_b):
    import ml_dtypes

    q = np.asarray(q, dtype=np.float32)
    k = np.asarray(k, dtype=np.float32)
    v = np.asarray(v, dtype=np.float32)
    am = np.asarray(attention_mask)

    use_pad = not bool((am == 1).all())
    use_vbias = bool(np.any(np.asarray(Wv_b) != 0))
    use_qkbias = bool(np.any(np.asarray(Wq_b) != 0) or np.any(np.asarray(Wk_b) != 0))

    perms = _perms()

    if use_qkbias or use_pad or use_vbias:
        return _kernel_general(q, k, v, am, Wq_w, Wq_b, Wk_w, Wk_b, Wv_w,
                               Wv_b, use_pad, use_vbias, perms)

    f8 = ml_dtypes.float8_e4m3
    bf = ml_dtypes.bfloat16

    nc = _build_fast(FP8_CHUNK1)

    def pmajor(x):
        """[MC*P, cols] -> [P, MC, cols]: [p, mc, col] = x[mc*128+p, col]."""
        mc = x.shape[0] // P
        return np.ascontiguousarray(
            x.reshape(mc, P, x.shape[1]).transpose(1, 0, 2))

    A32 = (np.asarray(Wq_w, np.float64).T @ np.asarray(Wk_w, np.float64))
    A32 = np.ascontiguousarray((A32 * 32.0).astype(np.float32))
    # a16f[hb, p, mc, j] = A32[mc*128+p, hb*128+j]: per-hb contiguous slices
    a16f = np.ascontiguousarray(
        A32.reshape(8, P, 8, P).transpose(2, 1, 0, 3)).astype(bf)
    a16h0 = np.ascontiguousarray(a16f[0])
    a16 = np.ascontiguousarray(a16f[1:])
    a8 = pmajor(A32).astype(f8)
    wvt16 = pmajor(np.ascontiguousarray(
        (np.asarray(Wv_w, np.float32).T * 32.0))).astype(bf)

    mask_b, mask_f = _masks(ml_dtypes)
    # masks to [p, i, n]
    mask_b = [np.ascontiguousarray(m.transpose(1, 0, 2)) for m in mask_b]
    mask_f = [np.ascontiguousarray(m.transpose(1, 0, 2)) for m in mask_f]

    nbf = 512 if FP8_CHUNK1 else 1024
    in_maps = []
    for cid in range(N_CORES):
        b, c = cid // 2, cid % 2
        qT = np.ascontiguousarray(q[b].T[:, perms[c]])
        kT = np.ascontiguousarray(k[b].T)
        vkb = v[b].reshape(NB, P, MD).transpose(1, 0, 2)  # [p, kb, m]
        m = dict(
            a16h0=a16h0, a16=a16, a8=a8,
            qt16=pmajor(qT[:, 0:512]).astype(bf),
            qt8=pmajor(qT[:, 512:1024]).astype(f8),
            kt16=pmajor(kT[:, 0:nbf]).astype(bf),
            kt8=pmajor(kT).astype(f8),
            v16=np.ascontiguousarray(vkb[:, 0:nbf // P, :]).astype(bf),
            v8=np.ascontiguousarray(vkb).astype(f8),
            wvt16=wvt16,
            m16=mask_b[c], m32=mask_f[c],
        )
        in_maps.append(m)

    res = _run(nc, in_maps)

    out = np.empty((B, S, HD), np.float32)
    for cid in range(N_CORES):
        b, c = cid // 2, cid % 2
        out[b, perms[c], :] = res.results[cid]["out"]
    return out


def _kernel_general(q, k, v, am, Wq_w, Wq_b, Wk_w, Wk_b, Wv_w, Wv_b,
                    use_pad, use_vbias, perms):
    nc = _build_general(use_pad, use_vbias)

    wqt = np.ascontiguousarray(np.asarray(Wq_w, np.float32).T)
    wkt = np.ascontiguousarray(np.asarray(Wk_w, np.float32).T)
    wvt = np.ascontiguousarray(np.asarray(Wv_w, np.float32).T)
    bq = np.ascontiguousarray(np.asarray(Wq_b, np.float32))
    bk = np.ascontiguousarray(np.asarray(Wk_b, np.float32))
    bv = np.ascontiguousarray(np.asarray(Wv_b, np.float32))

    import ml_dtypes
    mask_b, _ = _masks(ml_dtypes)

    kT = [np.ascontiguousarray(k[b].T) for b in range(B)]
    vT = [np.ascontiguousarray(v[b].T) for b in range(B)]

    in_maps = []
    for cid in range(N_CORES):
        b, c = cid // 2, cid % 2
        qT = np.ascontiguousarray(q[b].T[:, perms[c]])
        m = dict(qt=qT, kt=kT[b], vt=vT[b], wqt=wqt, wkt=wkt, wvt=wvt,
                 bq=bq, bk=bk, masks=mask_b[c])
        if use_pad:
            padv = am[b].astype(np.float32)
            m["padm"] = np.ascontiguousarray(padv.reshape(NB, P).T)
        if use_vbias:
            m["bv"] = bv
        in_maps.append(m)

    res = _run(nc, in_maps)

    out = np.empty((B, S, HD), np.float32)
    for cid in range(N_CORES):
        b, c = cid // 2, cid % 2
        out[b, perms[c], :] = res.results[cid]["out"]
    return out


# revision 43
# speedup vs baseline: 1.3298x; 1.1423x over previous
"""Single-head causal attention (B=4, S=2048, M=H=1024) on 8 Trainium2 cores.

Sharding: core = (batch, half). Each core handles one batch and half its
queries. To balance the causal triangle, query 128-blocks are interleaved
stride-2: core half c owns global q-blocks {c, c+2, ..., c+14}, grouped in
4 chunks of 256 queries; chunk j = global blocks {4j+c, 4j+c+2} and attends
key blocks [0, 4j+4) — the last 4 get data-driven causal masks, so the one
compiled program serves both halves (SPMD).

Fast path (zero qk-bias, no padding, zero v-bias) math, with A = Wq.T@Wk
folded on host (scaled by 32 so fp8/bf16 operands are ~unit variance):
  qh[m2, sq] = (32A).T @ qT          (bf16 matmul; fp8 for query cols 512:)
  scoresT[sk, sq] = ktT.T @ qh       (bf16 chunks 0-1, fp8-DoubleRow 2-3)
  e = exp(scoresT/1024 [- 2])        (ACT; fp8 chunks get -2 bias, cancels)
  Y[m, sq] = sum_kb v_kb.T @ e_kb    (late V-proj: raw v, no projection!)
  out[sq, h] = (Y.T @ (32Wv.T)) / (32*den),  den = sum_k e
The V projection is algebraically moved AFTER the attention-weighted sum,
so the big S*M*H projection runs once per core on [256-query, 1024] Y tiles
instead of all 2048 keys (the old kernel projected all of V on both halves
of every batch - pure duplicated work).

fp8 use is per-query-chunk: early queries attend few keys, so fp8 noise in
their softmax doesn't average out; late chunks attend >=512 keys and the
1/sqrt(n_eff) averaging makes fp8 safe (verified vs reference on host).
"""

import os

import numpy as np

B, S, MD, HD = 4, 2048, 1024, 1024
P = 128
NB = S // P            # 16 key/query blocks per batch
NCH = 4                # q-chunks of 256 per core
SQL = S // 2           # 1024 local queries per core
N_CORES = 8

FP8_CHUNK1 = True      # extend fp8 scores/Y to chunk 1 (queries 256:512)


def _build_fast(fp8_chunk1: bool):
    import concourse.bacc as bacc
    import concourse.mybir as mybir
    import concourse.tile as tile

    f32 = mybir.dt.float32
    bf16 = mybir.dt.bfloat16
    f8 = mybir.dt.float8e4
    Act = mybir.ActivationFunctionType
    DR = mybir.MatmulPerfMode.DoubleRow

    nc = bacc.Bacc("TRN2", num_swdge_queues=4, dynamic_dma_scratch_size=2048)

    nbf = 512 if fp8_chunk1 else 1024  # bf16 key coverage (chunks 0[,1])
    MC = MD // P   # 8 contraction chunks
    NKB = nbf // P
    # All inputs are pre-rearranged on host to partition-major layout, so
    # every load is a plain 2D copy: 128 big contiguous descriptors instead
    # of 1024+ small ones (8x less DMA issue time).
    a16h0 = nc.dram_tensor("a16h0", [P, MC, P], bf16, kind="ExternalInput")
    a16 = nc.dram_tensor("a16", [MC - 1, P, MC, P], bf16,
                         kind="ExternalInput")
    a8 = nc.dram_tensor("a8", [P, MC, HD], f8, kind="ExternalInput")
    qt16 = nc.dram_tensor("qt16", [P, MC, 512], bf16, kind="ExternalInput")
    qt8 = nc.dram_tensor("qt8", [P, MC, 512], f8, kind="ExternalInput")
    kt16 = nc.dram_tensor("kt16", [P, MC, nbf], bf16, kind="ExternalInput")
    kt8 = nc.dram_tensor("kt8", [P, MC, S], f8, kind="ExternalInput")
    v16 = nc.dram_tensor("v16", [P, NKB, MD], bf16, kind="ExternalInput")
    v8 = nc.dram_tensor("v8", [P, NB, MD], f8, kind="ExternalInput")
    wvt16 = nc.dram_tensor("wvt16", [P, MC, HD], bf16, kind="ExternalInput")
    m16 = nc.dram_tensor("m16", [P, 4, 256], bf16, kind="ExternalInput")
    m32 = nc.dram_tensor("m32", [P, 4, 256], f32, kind="ExternalInput")
    out = nc.dram_tensor("out", [SQL, HD], f32, kind="ExternalOutput")

    with tile.TileContext(nc) as tc:
        with (
            tc.tile_pool(name="res", bufs=1) as res,
            tc.tile_pool(name="exp", bufs=10) as epool,
            tc.tile_pool(name="ysb", bufs=2) as ypool,
            tc.tile_pool(name="outp", bufs=2) as outp,
            tc.tile_pool(name="small", bufs=6) as small,
            # PSUM: 8 banks total, every tile is bank-rounded. "y" is a shared
            # ring for q-proj psums, Y-accumulation passes and out-proj tiles.
            tc.tile_pool(name="yp", bufs=4, space="PSUM") as ypp,
            tc.tile_pool(name="sc", bufs=2, space="PSUM") as scp,
            tc.tile_pool(name="dn", bufs=2, space="PSUM") as dnp,
        ):
            # ---- resident tiles + DMA kickoff ----
            # Criticals first on each queue; a16 arrives as per-hb column
            # slices so the first q-proj psum only waits on ~0.75MB. The big
            # fp8 bulk (kt8/v8) sits on the otherwise-idle gpsimd queue
            # behind a tiny SBUF->SBUF DMA that depends on the first q-proj
            # copy, so it cannot crowd the startup-critical transfers.
            # critical path: qt16 + the hb=0 slice of a16 (0.75MB on scalar)
            qt16_t = res.tile([P, MC, 512], bf16, tag="qt16")
            nc.scalar.dma_start(qt16_t[:], qt16.ap())
            a16_t = res.tile([P, MC, HD], bf16, tag="a16")
            nc.scalar.dma_start(a16_t[:, :, 0:P], a16h0.ap())

            qt8_t = res.tile([P, MC, 512], f8, tag="qt8")
            nc.gpsimd.dma_start(qt8_t[:], qt8.ap())

            mt16 = res.tile([P, 4, 256], bf16, tag="mt16")
            nc.gpsimd.dma_start(mt16[:], m16.ap())
            mt32 = res.tile([P, 4, 256], f32, tag="mt32")
            nc.gpsimd.dma_start(mt32[:], m32.ap())

            v16_t = res.tile([P, NKB, MD], bf16, tag="v16")
            nc.gpsimd.dma_start(v16_t[:], v16.ap())

            # Remaining resident tiles: loads emitted inside phase 1a (one
            # iteration ahead of use) so the scheduler cannot couple the
            # first matmul chains' semaphore waits to them; bulk loads are
            # additionally serialized behind criticals via tiny SBUF->SBUF
            # "gate" writes into their own target tiles (WAW data deps);
            # the real full-tile load then overwrites the gate bytes.
            kt16_t = res.tile([P, MC, nbf], bf16, tag="kt16")
            a8_t = res.tile([P, MC, HD], f8, tag="a8")
            wv_t = res.tile([P, MC, HD], bf16, tag="wv")
            kt8_t = res.tile([P, MC, S], f8, tag="kt8")
            v8_t = res.tile([P, NB, MD], f8, tag="v8")

            ones16 = res.tile([P, 2], bf16, tag="ones16")
            nc.vector.memset(ones16[:], 32.0)
            ones8 = res.tile([P, 2, 2], f8, tag="ones8")
            nc.vector.memset(ones8[:], 32.0)
            nbias = res.tile([P, 1], f32, tag="nbias")
            nc.vector.memset(nbias[:], -2.0)

            qh16 = res.tile([P, MC, 512], bf16, tag="qh16")
            qh8 = res.tile([P, MC, 512], f8, tag="qh8")
            if fp8_chunk1:
                qh8c1 = res.tile([P, MC, 256], f8, tag="qh8c1")

            # ---- phase 1a: qh cols 0:512 (bf16) ----
            for hb in range(MC):
                if hb + 1 < MC:  # prefetch next a16 slice
                    h2 = hb + 1
                    nc.sync.dma_start(
                        a16_t[:, :, h2 * P:(h2 + 1) * P], a16.ap()[h2 - 1])
                ps = ypp.tile([P, 512], f32, tag="y", name=f"q16_{hb}")
                for mc in range(MC):
                    nc.tensor.matmul(
                        ps[:], a16_t[:, mc, hb * P:(hb + 1) * P],
                        qt16_t[:, mc, :],
                        start=(mc == 0), stop=(mc == MC - 1))
                nc.vector.tensor_copy(qh16[:, hb, :], ps[:])
                if fp8_chunk1:
                    nc.scalar.copy(qh8c1[:, hb, :], ps[:, 256:512])
                if hb == 0:
                    # gate fp8 bulk behind the first q-proj copy (real dep),
                    # and v8 behind the whole kt8 transfer
                    gsrc = qh8c1 if fp8_chunk1 else qh8
                    nc.gpsimd.dma_start(kt8_t[:, 0, 0:2], gsrc[:, 0, 0:2])
                    nc.gpsimd.dma_start(kt8_t[:], kt8.ap())
                    nc.gpsimd.dma_start(v8_t[:, 0, 0:2], kt8_t[:, 0, 0:2])
                    nc.gpsimd.dma_start(v8_t[:], v8.ap())
                if hb == 2:
                    nc.scalar.dma_start(kt16_t[:, 0, 0:2], qt16_t[:, 0, 0:2])
                    nc.scalar.dma_start(kt16_t[:], kt16.ap())
                    nc.scalar.dma_start(a8_t[:, 0, 0:2], qt8_t[:, 0, 0:2])
                    nc.scalar.dma_start(a8_t[:], a8.ap())
                if hb == 4:
                    nc.sync.dma_start(wv_t[:, 0, 0:2], kt16_t[:, 0, 0:2])
                    nc.sync.dma_start(wv_t[:], wvt16.ap())

            # ---- phase 1b: qh cols 512:1024 (fp8 DoubleRow) ----
            for hb in range(MC):
                ps = ypp.tile([P, 512], f32, tag="y", name=f"q8_{hb}")
                for i in range(4):
                    nc.tensor.matmul(
                        ps[:], a8_t[:, 2 * i:2 * i + 2, hb * P:(hb + 1) * P],
                        qt8_t[:, 2 * i:2 * i + 2, :],
                        start=(i == 0), stop=(i == 3), perf_mode=DR)
                nc.scalar.copy(qh8[:, hb, :], ps[:])

            # ---- phase 2: attention chunks ----
            pending = None  # (ysb, col-offset, rr[2], j) awaiting out-proj

            def emit_out_proj(p):
                ysb, off, drs, j = p
                for t in range(2):
                    ops = [ypp.tile([P, 512], f32, tag="y",
                                    name=f"op{j}_{t}_{hc}") for hc in range(2)]
                    for mb in range(MC):
                        for hc in range(2):
                            nc.tensor.matmul(
                                ops[hc][:],
                                ysb[:, mb, off + t * P:off + (t + 1) * P],
                                wv_t[:, mb, hc * 512:(hc + 1) * 512],
                                start=(mb == 0), stop=(mb == MC - 1))
                    rr = drs[t][:, 1:2]
                    o = outp.tile([P, HD], f32, tag="o")
                    lb = 2 * j + t
                    nc.vector.tensor_scalar_mul(o[:, 0:512], ops[0][:], rr[:])
                    nc.sync.dma_start(out.ap()[lb * P:(lb + 1) * P, 0:512],
                                      o[:, 0:512])
                    nc.scalar.activation(o[:, 512:1024], ops[1][:], Act.Copy,
                                         scale=rr[:])
                    nc.sync.dma_start(out.ap()[lb * P:(lb + 1) * P, 512:1024],
                                      o[:, 512:1024])

            for j in range(2):
                E = 4 * j + 4
                use8 = (j >= 2) or (j == 1 and fp8_chunk1)
                sq0 = (j % 2) * 256  # col offset within qh16/qh8 halves
                dns = [dnp.tile([P, 2], f32, tag="d", name=f"d{j}_{t}")
                       for t in range(2)]
                exps = []

                if not use8:
                    qrhs = qh16[:, :, sq0:sq0 + 256]
                    for kb in range(E):
                        sps = scp.tile([P, 256], f32, tag="s")
                        for mc in range(MC):
                            nc.tensor.matmul(
                                sps[:], kt16_t[:, mc, kb * P:(kb + 1) * P],
                                qrhs[:, mc, :],
                                start=(mc == 0), stop=(mc == MC - 1))
                        ex = epool.tile([P, 256], bf16, tag="e")
                        nc.scalar.activation(ex[:], sps[:], Act.Exp,
                                             scale=1.0 / 1024.0)
                        if kb >= 4 * j:
                            nc.vector.tensor_mul(ex[:], ex[:],
                                                 mt16[:, kb - 4 * j, :])
                        for t in range(2):
                            nc.tensor.matmul(
                                dns[t][:], ex[:, t * P:(t + 1) * P],
                                ones16[:], start=(kb == 0), stop=(kb == E - 1))
                        exps.append(ex)
                        if kb == 1 and pending is not None:
                            emit_out_proj(pending)
                            pending = None
                else:
                    if j == 1:
                        qrhs = qh8c1
                    else:
                        qrhs = qh8[:, :, sq0:sq0 + 256]
                    NP = E // 2
                    for p in range(NP):
                        e8p = epool.tile([P, 2, 256], f8, tag="e8")
                        for s2 in range(2):
                            kb = 2 * p + s2
                            sps = scp.tile([P, 256], f32, tag="s")
                            for i in range(4):
                                nc.tensor.matmul(
                                    sps[:],
                                    kt8_t[:, 2 * i:2 * i + 2,
                                          kb * P:(kb + 1) * P],
                                    qrhs[:, 2 * i:2 * i + 2, :],
                                    start=(i == 0), stop=(i == 3),
                                    perf_mode=DR)
                            if kb >= 4 * j:
                                nc.vector.tensor_add(sps[:], sps[:],
                                                     mt32[:, kb - 4 * j, :])
                            nc.scalar.activation(e8p[:, s2, :], sps[:],
                                                 Act.Exp, scale=1.0 / 1024.0,
                                                 bias=nbias[:])
                        for t in range(2):
                            nc.tensor.matmul(
                                dns[t][:], e8p[:, :, t * P:(t + 1) * P],
                                ones8[:], start=(p == 0), stop=(p == NP - 1),
                                perf_mode=DR)
                        exps.append(e8p)
                        if p == 0 and pending is not None:
                            emit_out_proj(pending)
                            pending = None

                # denominators -> reciprocal (frees dn ring before Y passes)
                drs = []
                for t in range(2):
                    dr = small.tile([P, 2], f32, tag="dr")
                    nc.vector.tensor_copy(dr[:, 0:1], dns[t][:, 0:1])
                    nc.vector.reciprocal(dr[:, 1:2], dr[:, 0:1])
                    drs.append(dr)

                # Y accumulation in 4 passes of 2 m-blocks (PSUM bank limit)
                ysb = ypool.tile([P, MC, 256], bf16, tag="ysb")
                for d in range(4):
                    ys2 = [ypp.tile([P, 256], f32, tag="y",
                                    name=f"y{j}_{d}_{i}") for i in range(2)]
                    for i in range(2):
                        mb = 2 * d + i
                        if not use8:
                            for kb in range(E):
                                nc.tensor.matmul(
                                    ys2[i][:],
                                    v16_t[:, kb, mb * P:(mb + 1) * P],
                                    exps[kb][:],
                                    start=(kb == 0), stop=(kb == E - 1))
                        else:
                            NP = E // 2
                            for p in range(NP):
                                nc.tensor.matmul(
                                    ys2[i][:],
                                    v8_t[:, 2 * p:2 * p + 2,
                                         mb * P:(mb + 1) * P],
                                    exps[p][:],
                                    start=(p == 0), stop=(p == NP - 1),
                                    perf_mode=DR)
                        if mb % 2 == 0:
                            nc.vector.tensor_copy(ysb[:, mb, :], ys2[i][:])
                        else:
                            nc.scalar.copy(ysb[:, mb, :], ys2[i][:])
                pending = (ysb, 0, drs, j)

            # ---- merged chunks 2 & 3 (both fp8, local queries 512:1024) ----
            # Shared 512-wide scores for kb<12 (both chunks attend them) and
            # one shared Y accumulation in [128,512] psums; kb 12-15 are
            # chunk-3-only and accumulate into the upper half.
            exps23 = []
            for p in range(8):
                wide = p < 6
                cols = 512 if wide else 256
                et = epool.tile([P, 2, cols], f8,
                                tag="e8w" if wide else "e8", name=f"e23_{p}")
                for s2 in range(2):
                    kb = 2 * p + s2
                    sps = scp.tile([P, cols], f32, tag="s", name=f"s23_{kb}")
                    qrhs = qh8[:, :, 0:512] if wide else qh8[:, :, 256:512]
                    for i in range(4):
                        nc.tensor.matmul(
                            sps[:], kt8_t[:, 2 * i:2 * i + 2,
                                          kb * P:(kb + 1) * P],
                            qrhs[:, 2 * i:2 * i + 2, :],
                            start=(i == 0), stop=(i == 3), perf_mode=DR)
                    if wide and kb >= 8:
                        nc.vector.tensor_add(sps[:, 0:256], sps[:, 0:256],
                                             mt32[:, kb - 8, :])
                    if not wide:
                        nc.vector.tensor_add(sps[:], sps[:],
                                             mt32[:, kb - 12, :])
                    nc.scalar.activation(et[:, s2, :], sps[:], Act.Exp,
                                         scale=1.0 / 1024.0, bias=nbias[:])
                exps23.append(et)
                if p == 0 and pending is not None:
                    emit_out_proj(pending)
                    pending = None

            # denominators: chunk2 over pairs 0-5 (cols 0:256), chunk3 over
            # all pairs (cols 256:512 of wide tiles, 0:256 of narrow)
            all_drs = []
            for cj, off, NPd in ((2, 256, 6), (3, 0, 8)):
                dns = [dnp.tile([P, 2], f32, tag="d", name=f"d{cj}_{t}")
                       for t in range(2)]
                for p in range(NPd if cj == 2 else 8):
                    for t in range(2):
                        if cj == 2:
                            lhs = exps23[p][:, :, t * P:(t + 1) * P]
                        elif p < 6:
                            lhs = exps23[p][:, :, 256 + t * P:256 + (t + 1) * P]
                        else:
                            lhs = exps23[p][:, :, t * P:(t + 1) * P]
                        nc.tensor.matmul(
                            dns[t][:], lhs, ones8[:],
                            start=(p == 0),
                            stop=(p == (5 if cj == 2 else 7)),
                            perf_mode=DR)
                drs = []
                for t in range(2):
                    dr = small.tile([P, 2], f32, tag="dr")
                    nc.vector.tensor_copy(dr[:, 0:1], dns[t][:, 0:1])
                    nc.vector.reciprocal(dr[:, 1:2], dr[:, 0:1])
                    drs.append(dr)
                all_drs.append(drs)

            # shared Y accumulation: full-width for pairs 0-5, upper half
            # only for the chunk-3 tail pairs 6-7
            ysb23 = ypool.tile([P, MC, 512], bf16, tag="ysb")
            for d in range(4):
                ys2 = [ypp.tile([P, 512], f32, tag="y",
                                name=f"y23_{d}_{i}") for i in range(2)]
                for i in range(2):
                    mb = 2 * d + i
                    for p in range(8):
                        if p < 6:
                            nc.tensor.matmul(
                                ys2[i][:],
                                v8_t[:, 2 * p:2 * p + 2, mb * P:(mb + 1) * P],
                                exps23[p][:], start=(p == 0), stop=False,
                                perf_mode=DR, skip_group_check=True)
                        else:
                            nc.tensor.matmul(
                                ys2[i][:, 256:512],
                                v8_t[:, 2 * p:2 * p + 2, mb * P:(mb + 1) * P],
                                exps23[p][:], start=False, stop=(p == 7),
                                perf_mode=DR, skip_group_check=True)
                    if mb % 2 == 0:
                        nc.vector.tensor_copy(ysb23[:, mb, :], ys2[i][:])
                    else:
                        nc.scalar.copy(ysb23[:, mb, :], ys2[i][:])

            emit_out_proj((ysb23, 0, all_drs[0], 2))
            emit_out_proj((ysb23, 256, all_drs[1], 3))

    nc.compile()
    return nc


def _build_general(use_pad: bool, use_vbias: bool):
    import concourse.bacc as bacc
    import concourse.mybir as mybir
    import concourse.tile as tile

    f32 = mybir.dt.float32
    f32r = mybir.dt.float32r
    bf16 = mybir.dt.bfloat16
    Act = mybir.ActivationFunctionType

    nc = bacc.Bacc("TRN2", num_swdge_queues=4, dynamic_dma_scratch_size=2048)

    qt = nc.dram_tensor("qt", [MD, SQL], f32r, kind="ExternalInput")
    kt = nc.dram_tensor("kt", [MD, S], f32r, kind="ExternalInput")
    vt = nc.dram_tensor("vt", [MD, S], f32r, kind="ExternalInput")
    wqt = nc.dram_tensor("wqt", [MD, HD], f32r, kind="ExternalInput")
    wkt = nc.dram_tensor("wkt", [MD, HD], f32r, kind="ExternalInput")
    wvt = nc.dram_tensor("wvt", [MD, HD], f32r, kind="ExternalInput")
    bq = nc.dram_tensor("bq", [HD], f32, kind="ExternalInput")
    bk = nc.dram_tensor("bk", [HD], f32, kind="ExternalInput")
    masks = nc.dram_tensor("masks", [4, P, 256], bf16, kind="ExternalInput")
    if use_pad:
        padm = nc.dram_tensor("padm", [P, NB], f32, kind="ExternalInput")
    if use_vbias:
        bv = nc.dram_tensor("bv", [HD], f32, kind="ExternalInput")
    out = nc.dram_tensor("out", [SQL, HD], f32, kind="ExternalOutput")

    MC = MD // P   # 8 contraction chunks
    HB = HD // P   # 8 h-blocks (partition dim of qhT/khT)

    with tile.TileContext(nc) as tc:
        with (
            tc.tile_pool(name="res", bufs=1) as res,
            tc.tile_pool(name="w", bufs=10) as wpool,
            tc.tile_pool(name="xin", bufs=4) as xin,
            tc.tile_pool(name="exp", bufs=16) as epool,
            tc.tile_pool(name="outp", bufs=1) as outp,
            tc.tile_pool(name="small", bufs=2) as small,
            tc.tile_pool(name="mm", bufs=5, space="PSUM") as mmp,
            tc.tile_pool(name="sc", bufs=2, space="PSUM") as scp,
            tc.tile_pool(name="dn", bufs=1, space="PSUM") as dnp,
        ):
            qh = res.tile([P, HB, SQL], f32r, tag="qh")
            kh = res.tile([P, HB, S], f32r, tag="kh")
            vh = res.tile([P, NB, HD], bf16, tag="vh")
            mt = res.tile([P, 4, 256], bf16, tag="mt")
            nc.scalar.dma_start(mt[:], masks.ap().rearrange("i p n -> p i n"))
            ones = res.tile([P, 2], bf16, tag="ones")
            nc.vector.memset(ones[:], 1.0)
            bias_t = res.tile([P, 2 * HB], f32, tag="bias")
            bqt = bias_t[:, 0:HB]
            nc.gpsimd.dma_start(bqt[:], bq.ap().rearrange("(hb p) -> p hb", p=P))
            bkt = bias_t[:, HB:2 * HB]
            nc.gpsimd.dma_start(bkt[:], bk.ap().rearrange("(hb p) -> p hb", p=P))
            if use_pad:
                pad_t = res.tile([P, NB], f32, tag="pad")
                nc.gpsimd.dma_start(pad_t[:], padm.ap())
            if use_vbias:
                ones_row = res.tile([1, P], f32r, tag="or")
                bvr = res.tile([1, HD], f32r, tag="bvr")
                nc.gpsimd.memset(ones_row[:].bitcast(f32), 1.0)
                nc.gpsimd.dma_start(bvr[:], bv.ap()[None, :])

            def load_w(dram, split=True):
                tiles = []
                for mc in range(MC):
                    t = wpool.tile([P, HD], f32r, tag="w", name=f"w{mc}")
                    weng = nc.scalar if (mc % 2 == 0 or not split) else nc.sync
                    weng.dma_start(t[:], dram.ap()[mc * P:(mc + 1) * P, :])
                    tiles.append(t)
                return tiles

            class XPair:
                def __init__(self, a, b):
                    self.a, self.b = a, b

                def __getitem__(self, key):
                    _, mc, cols = key
                    t = self.a if mc < 4 else self.b
                    return t[:, mc % 4, cols]

            def load_x(dram, c0):
                r = dram.ap().rearrange("(mc p) s -> p mc s", p=P)
                a = xin.tile([P, 4, 512], f32r, tag="x", name="xa")
                nc.sync.dma_start(a[:], r[:, 0:4, c0:c0 + 512])
                b = xin.tile([P, 4, 512], f32r, tag="x", name="xb")
                nc.sync.dma_start(b[:], r[:, 4:8, c0:c0 + 512])
                return XPair(a, b)

            # ---- Q projection: qh[:, hb, sq] (h on partitions) ----
            wq_t = load_w(wqt, split=False)
            for sqc in range(SQL // 512):
                xts = load_x(qt, sqc * 512)
                for hb in range(HB):
                    ps = mmp.tile([P, 512], f32, tag="mm")
                    for mc in range(MC):
                        nc.tensor.matmul(
                            ps[:], wq_t[mc][:, hb * P:(hb + 1) * P], xts[:, mc, :],
                            start=(mc == 0), stop=(mc == MC - 1))
                    nc.vector.tensor_scalar_add(
                        qh[:, hb, sqc * 512:(sqc + 1) * 512], ps[:],
                        bqt[:, hb:hb + 1])

            # ---- K projection: kh[:, hb, sk] ----
            wk_t = load_w(wkt)
            for skc in range(S // 512):
                xts = load_x(kt, skc * 512)
                for hb in range(HB):
                    ps = mmp.tile([P, 512], f32, tag="mm")
                    for mc in range(MC):
                        nc.tensor.matmul(
                            ps[:], wk_t[mc][:, hb * P:(hb + 1) * P], xts[:, mc, :],
                            start=(mc == 0), stop=(mc == MC - 1))
                    nc.vector.tensor_scalar_add(
                        kh[:, hb, skc * 512:(skc + 1) * 512], ps[:],
                        bkt[:, hb:hb + 1])

            # ---- V projection: vh[:, skb, h] (keys on partitions) ----
            wv_t = load_w(wvt)
            for skc in range(S // 512):
                xts = load_x(vt, skc * 512)
                for sbl in range(4):
                    skb = skc * 4 + sbl
                    for hc in range(2):
                        ps = mmp.tile([P, 512], f32, tag="mm")
                        for mc in range(MC):
                            nc.tensor.matmul(
                                ps[:], xts[:, mc, sbl * P:(sbl + 1) * P],
                                wv_t[mc][:, hc * 512:(hc + 1) * 512],
                                start=(mc == 0),
                                stop=(mc == MC - 1) and not use_vbias)
                        if use_vbias:
                            nc.tensor.matmul(
                                ps[:], ones_row[:],
                                bvr[:, hc * 512:(hc + 1) * 512],
                                start=False, stop=True)
                        nc.vector.tensor_copy(vh[:, skb, hc * 512:(hc + 1) * 512], ps[:])


            # ---- attention, chunk j = 256 queries, keys [0, (4j+4)*128) ----
            for j in range(NCH):
                E = 4 * j + 4
                sq0 = j * 256
                exps = []
                for kb in range(E):
                    sps = scp.tile([P, 256], f32, tag="s")
                    for hb in range(HB):
                        nc.tensor.matmul(
                            sps[:], kh[:, hb, kb * P:(kb + 1) * P],
                            qh[:, hb, sq0:sq0 + 256],
                            start=(hb == 0), stop=(hb == HB - 1))
                    ex = epool.tile([P, 256], bf16, tag="e")
                    nc.scalar.activation(ex[:], sps[:], Act.Exp, scale=1.0 / 32.0)
                    if kb >= 4 * j:
                        nc.vector.tensor_mul(ex[:], ex[:], mt[:, kb - 4 * j, :])
                    if use_pad:
                        nc.vector.tensor_scalar_mul(ex[:], ex[:], pad_t[:, kb:kb + 1])
                    exps.append(ex)

                for t in range(2):
                    dps = dnp.tile([P, 2], f32, tag="d")
                    avs = [mmp.tile([P, 512], f32, tag="mm", name=f"av{j}_{t}_{hc2}")
                           for hc2 in range(2)]
                    for kb in range(E):
                        lhs = exps[kb][:, t * P:(t + 1) * P]
                        for hc in range(2):
                            nc.tensor.matmul(
                                avs[hc][:], lhs, vh[:, kb, hc * 512:(hc + 1) * 512],
                                start=(kb == 0), stop=(kb == E - 1))
                        nc.tensor.matmul(
                            dps[:], lhs, ones[:],
                            start=(kb == 0), stop=(kb == E - 1))
                    dr = small.tile([P, 2], f32, tag="dr")
                    nc.vector.tensor_copy(dr[:, 0:1], dps[:, 0:1])
                    rr = dr[:, 1:2]
                    nc.vector.reciprocal(rr[:], dr[:, 0:1])
                    o = outp.tile([P, HD], f32, tag="o")
                    for hc in range(2):
                        nc.vector.tensor_scalar_mul(
                            o[:, hc * 512:(hc + 1) * 512], avs[hc][:], rr[:])
                    lb = 2 * j + t
                    nc.sync.dma_start(out.ap()[lb * P:(lb + 1) * P, :], o[:])

    nc.compile()
    return nc


def _ntff_hook():
    """NTFF profile hook via direct ctypes into libaxon_pjrt.so (the
    antenv.axon_hooks module is absent in this image). Dev-only: guarded
    by ATTN_PROF_DIR in kernel(); the grading path never reaches this."""
    import contextlib
    import ctypes
    import sys

    lib = ctypes.CDLL("/opt/axon/libaxon_pjrt.so")
    lib.axon_start_nrt_profile.argtypes = [
        ctypes.POINTER(ctypes.c_int64), ctypes.c_size_t]
    lib.axon_start_nrt_profile.restype = ctypes.c_int64
    lib.axon_stop_nrt_profile.argtypes = [ctypes.c_char_p]
    lib.axon_stop_nrt_profile.restype = ctypes.c_int64

    @contextlib.contextmanager
    def _hook(output_dir, device_ids):
        import jax
        jax.devices()
        if device_ids:
            ids = (ctypes.c_int64 * len(device_ids))(*device_ids)
            rc = lib.axon_start_nrt_profile(ids, len(device_ids))
        else:
            rc = lib.axon_start_nrt_profile(None, 0)
        if rc != 0:
            raise RuntimeError(f"axon_start_nrt_profile rc={rc}")
        try:
            yield
        finally:
            n = lib.axon_stop_nrt_profile(str(output_dir).encode())
            print(f"profile: {n} file(s) written to {output_dir}",
                  file=sys.stderr)

    return _hook


def _run(nc, in_maps):
    from concourse.bass_utils import run_bass_kernel_spmd

    prof_dir = os.environ.get("ATTN_PROF_DIR")
    if prof_dir:
        hook = _ntff_hook()
        with hook(prof_dir, [0]):
            return run_bass_kernel_spmd(nc, in_maps, list(range(N_CORES)))
    return run_bass_kernel_spmd(nc, in_maps, list(range(N_CORES)))


def _perms():
    perms = []
    for c in range(2):
        perm = np.concatenate([
            np.arange(P) + (4 * j + c + 2 * t) * P
            for j in range(NCH) for t in range(2)
        ])
        perms.append(perm)
    return perms


def _masks(ml_dtypes):
    """Causal masks for the 4 tail key-blocks of each chunk, per half c.
    entry [i, a, col]: key (4j+i)*128+a vs query (4j+c+2t)*128+b, t=col//128."""
    mask_b, mask_f = [], []
    a = np.arange(P)[:, None]
    col = np.arange(256)[None, :]
    for c in range(2):
        t = col // P
        b_ = col % P
        m = np.stack([
            (128 * i + a <= 128 * (c + 2 * t) + b_) for i in range(4)
        ])
        mask_b.append(m.astype(np.float32).astype(ml_dtypes.bfloat16))
        mask_f.append(np.where(m, np.float32(0), np.float32(-1e9)))
    return mask_b, mask_f


def kernel(q, k, v, attention_mask, Wq_w, Wq_b, Wk_w, Wk_b, Wv_w, Wv_b):
    import ml_dtypes

    q = np.asarray(q, dtype=np.float32)
    k = np.asarray(k, dtype=np.float32)
    v = np.asarray(v, dtype=np.float32)
    am = np.asarray(attention_mask)

    use_pad = not bool((am == 1).all())
    use_vbias = bool(np.any(np.asarray(Wv_b) != 0))
    use_qkbias = bool(np.any(np.asarray(Wq_b) != 0) or np.any(np.asarray(Wk_b) != 0))

    perms = _perms()

    if use_qkbias or use_pad or use_vbias:
        return _kernel_general(q, k, v, am, Wq_w, Wq_b, Wk_w, Wk_b, Wv_w,
                               Wv_b, use_pad, use_vbias, perms)

    f8 = ml_dtypes.float8_e4m3
    bf = ml_dtypes.bfloat16

    nc = _build_fast(FP8_CHUNK1)

    def pmajor(x):
        """[MC*P, cols] -> [P, MC, cols]: [p, mc, col] = x[mc*128+p, col]."""
        mc = x.shape[0] // P
        return np.ascontiguousarray(
            x.reshape(mc, P, x.shape[1]).transpose(1, 0, 2))

    A32 = (np.asarray(Wq_w, np.float64).T @ np.asarray(Wk_w, np.float64))
    A32 = np.ascontiguousarray((A32 * 32.0).astype(np.float32))
    # a16f[hb, p, mc, j] = A32[mc*128+p, hb*128+j]: per-hb contiguous slices
    a16f = np.ascontiguousarray(
        A32.reshape(8, P, 8, P).transpose(2, 1, 0, 3)).astype(bf)
    a16h0 = np.ascontiguousarray(a16f[0])
    a16 = np.ascontiguousarray(a16f[1:])
    a8 = pmajor(A32).astype(f8)
    wvt16 = pmajor(np.ascontiguousarray(
        (np.asarray(Wv_w, np.float32).T * 32.0))).astype(bf)

    mask_b, mask_f = _masks(ml_dtypes)
    # masks to [p, i, n]
    mask_b = [np.ascontiguousarray(m.transpose(1, 0, 2)) for m in mask_b]
    mask_f = [np.ascontiguousarray(m.transpose(1, 0, 2)) for m in mask_f]

    nbf = 512 if FP8_CHUNK1 else 1024
    in_maps = []
    for cid in range(N_CORES):
        b, c = cid // 2, cid % 2
        qT = np.ascontiguousarray(q[b].T[:, perms[c]])
        kT = np.ascontiguousarray(k[b].T)
        vkb = v[b].reshape(NB, P, MD).transpose(1, 0, 2)  # [p, kb, m]
        m = dict(
            a16h0=a16h0, a16=a16, a8=a8,
            qt16=pmajor(qT[:, 0:512]).astype(bf),
            qt8=pmajor(qT[:, 512:1024]).astype(f8),
            kt16=pmajor(kT[:, 0:nbf]).astype(bf),
            kt8=pmajor(kT).astype(f8),
            v16=np.ascontiguousarray(vkb[:, 0:nbf // P, :]).astype(bf),
            v8=np.ascontiguousarray(vkb).astype(f8),
            wvt16=wvt16,
            m16=mask_b[c], m32=mask_f[c],
        )
        in_maps.append(m)

    res = _run(nc, in_maps)

    out = np.empty((B, S, HD), np.float32)
    for cid in range(N_CORES):
        b, c = cid // 2, cid % 2
        out[b, perms[c], :] = res.results[cid]["out"]
    return out


def _kernel_general(q, k, v, am, Wq_w, Wq_b, Wk_w, Wk_b, Wv_w, Wv_b,
                    use_pad, use_vbias, perms):
    nc = _build_general(use_pad, use_vbias)

    wqt = np.ascontiguousarray(np.asarray(Wq_w, np.float32).T)
    wkt = np.ascontiguousarray(np.asarray(Wk_w, np.float32).T)
    wvt = np.ascontiguousarray(np.asarray(Wv_w, np.float32).T)
    bq = np.ascontiguousarray(np.asarray(Wq_b, np.float32))
    bk = np.ascontiguousarray(np.asarray(Wk_b, np.float32))
    bv = np.ascontiguousarray(np.asarray(Wv_b, np.float32))

    import ml_dtypes
    mask_b, _ = _masks(ml_dtypes)

    kT = [np.ascontiguousarray(k[b].T) for b in range(B)]
    vT = [np.ascontiguousarray(v[b].T) for b in range(B)]

    in_maps = []
    for cid in range(N_CORES):
        b, c = cid // 2, cid % 2
        qT = np.ascontiguousarray(q[b].T[:, perms[c]])
        m = dict(qt=qT, kt=kT[b], vt=vT[b], wqt=wqt, wkt=wkt, wvt=wvt,
                 bq=bq, bk=bk, masks=mask_b[c])
        if use_pad:
            padv = am[b].astype(np.float32)
            m["padm"] = np.ascontiguousarray(padv.reshape(NB, P).T)
        if use_vbias:
            m["bv"] = bv
        in_maps.append(m)

    res = _run(nc, in_maps)

    out = np.empty((B, S, HD), np.float32)
    for cid in range(N_CORES):
        b, c = cid // 2, cid % 2
        out[b, perms[c], :] = res.results[cid]["out"]
    return out
